# revision 1
# baseline (speedup 1.0000x reference)
# kernel.py — DiscriminativeLoss on 8 TRN2 NeuronCores (Bass/Tile, SPMD).
#
# Math (matches reference):
#   counts_k = #{i: l_i = k};  S_k = sum_{i in k} x_i;  mu_k = S_k / max(c_k, 1)
#   intra = (1/K) * sum_i invc_{l_i} * relu(||x_i - (mu - eps)|| - 1.5)^2
#   inter = sum_{a != b} relu(1 - ||(mu_a + eps) - mu_b||)^2 / (K*(K-1))
#   reg   = (1/K) * sum_k ||mu_k + eps||
#   total = intra + inter + 0.001 * reg
#
# Device strategy (per core, data-parallel over points):
#   - points-on-partitions layout: point i = p*TPC + j lives at [p, j]
#   - pass 1: one-hot H [128,64] per tile via DVE is_equal; PE matmul
#     lhsT=[X|1] [128,33] x rhs=H [128,64] accumulates [33,64] = [S^T; counts]
#   - AllReduce [33,64] across 8 cores
#   - stats: transpose -> [64,33]; mu, invc, inter/reg losses (tiny ops)
#   - pass 2: transposed one-hot HT [64,128] per tile (built from a host-
#     provided tile-major label copy via DMA broadcast + is_equal) used as
#     matmul WEIGHTS against Table [64,33]=[mu-eps | invc] -> per-point
#     gathered rows [128,33] (mu_{l_i}-eps, invc_{l_i})
#   - diff = x - (mu-eps); dist2 = rowsum(diff^2); hinge; dot with invc
#   - AllReduce intra partial; assemble scalar
import math
import numpy as np
from contextlib import ExitStack

import concourse.bass as bass
import concourse.bacc as bacc
import concourse.tile as tile
import concourse.mybir as mybir
from concourse.bass_utils import run_bass_kernel_spmd

F32 = mybir.dt.float32
BF16 = mybir.dt.bfloat16
I16 = mybir.dt.int16

N_CORES = 8
K = 64
D = 32
P = 128
EPS = 1e-8
PAD_LABEL = 999  # never matches any one-hot column

INTRA_MARGIN = 1.5
INTER_MARGIN2 = 1.0  # 2 * 0.5


def _host_prep(features, labels, tpc):
    """Shard + relayout on host. Returns per-core input dicts."""
    n_total = features.shape[0]
    n_core = n_total // N_CORES
    n_pad = P * tpc
    import ml_dtypes

    in_maps = []
    for c in range(N_CORES):
        f = np.asarray(features[c * n_core : (c + 1) * n_core], dtype=np.float32)
        l = np.asarray(labels[c * n_core : (c + 1) * n_core], dtype=np.int64)
        if n_pad > n_core:
            f = np.concatenate([f, np.zeros((n_pad - n_core, D), np.float32)], axis=0)
            l = np.concatenate([l, np.full((n_pad - n_core,), PAD_LABEL, np.int64)])
        # Xe: [P, tpc, 33] bf16, col 32 = 1.0
        xe = np.ones((n_pad, D + 1), np.float32)
        xe[:, :D] = f
        xe = xe.reshape(P, tpc, D + 1).astype(ml_dtypes.bfloat16)
        # p-major labels (for pass-1 one-hot): [P, tpc] int16, NO offset
        l_pm = l.reshape(P, tpc).astype(np.int16)
        # tile-major labels (for pass-2 transposed one-hot), paired:
        # A-set = tiles [0, na), B-set = tiles [na, tpc) with +64 offset.
        na = (tpc + 1) // 2
        ltm = l.reshape(P, tpc).T.astype(np.int16)  # [tpc, P]
        l_tma = np.ascontiguousarray(ltm[:na]).ravel()
        ltmb = np.full((na, P), PAD_LABEL, np.int16)
        ltmb[: tpc - na] = ltm[na:] + 64
        l_tmb = np.ascontiguousarray(ltmb).ravel()
        in_maps.append(
            {
                "xe": np.ascontiguousarray(xe),
                "labels_pm": np.ascontiguousarray(l_pm),
                "labels_tma": l_tma,
                "labels_tmb": l_tmb,
                "iota64": np.tile(np.arange(K, dtype=np.int16), (P, 1)),
                "labels_pmf": l_pm.astype(np.float32),
                "iota64f": np.tile(np.arange(K, dtype=np.float32), (P, 1)),
                "iotacol": np.arange(P, dtype=np.float32).reshape(P, 1),
                "id33": np.eye(D + 1, dtype=np.float32),
                "id64": np.eye(K, dtype=np.float32),
                "eyeneg": (1.0 - np.eye(K, dtype=np.float32)).astype(
                    ml_dtypes.bfloat16
                ),
            }
        )
    return in_maps


def build_program(tpc, j1=20, j2=8, stage=3):
    """Build the SPMD Bass program. tpc = tiles per core (cols per partition)."""
    nc = bacc.Bacc(
        "TRN2", target_bir_lowering=False, debug=False, num_devices=N_CORES
    )
    core_ids = list(range(N_CORES))

    xe_d = nc.dram_tensor("xe", [P, tpc, D + 1], BF16, kind="ExternalInput").ap()
    lpm_d = nc.dram_tensor("labels_pm", [P, tpc], I16, kind="ExternalInput").ap()
    lpmf_d = nc.dram_tensor("labels_pmf", [P, tpc], F32, kind="ExternalInput").ap()
    iota64f_d = nc.dram_tensor("iota64f", [P, K], F32, kind="ExternalInput").ap()
    na = (tpc + 1) // 2
    ltma_d = nc.dram_tensor("labels_tma", [na * P], I16, kind="ExternalInput").ap()
    ltmb_d = nc.dram_tensor("labels_tmb", [na * P], I16, kind="ExternalInput").ap()
    iota64_d = nc.dram_tensor("iota64", [P, K], I16, kind="ExternalInput").ap()
    iotacol_d = nc.dram_tensor("iotacol", [P, 1], F32, kind="ExternalInput").ap()
    id33_d = nc.dram_tensor("id33", [D + 1, D + 1], F32, kind="ExternalInput").ap()
    id64_d = nc.dram_tensor("id64", [K, K], F32, kind="ExternalInput").ap()
    eyeneg_d = nc.dram_tensor("eyeneg", [K, K], BF16, kind="ExternalInput").ap()
    out_d = nc.dram_tensor("out", [3], F32, kind="ExternalOutput").ap()

    with tile.TileContext(nc, num_cores=N_CORES) as tc, ExitStack() as ctx:
        singles = ctx.enter_context(tc.tile_pool(name="singles", bufs=1))
        xpool = ctx.enter_context(tc.tile_pool(name="xpool", bufs=1))
        hpool = ctx.enter_context(tc.tile_pool(name="hpool", bufs=4))
        htpool = ctx.enter_context(tc.tile_pool(name="htpool", bufs=4))
        l2pool = ctx.enter_context(tc.tile_pool(name="l2pool", bufs=4))
        wpool = ctx.enter_context(tc.tile_pool(name="wpool", bufs=3))
        psA = ctx.enter_context(tc.tile_pool(name="psA", bufs=1, space="PSUM"))
        psMg = ctx.enter_context(tc.tile_pool(name="psMg", bufs=4, space="PSUM"))
        psS = ctx.enter_context(tc.tile_pool(name="psS", bufs=3, space="PSUM"))
        dram = ctx.enter_context(tc.tile_pool(name="dram", bufs=2, space="DRAM"))

        # ---------- constants ----------
        iota64 = singles.tile([P, K], I16)
        nc.sync.dma_start(out=iota64, in_=iota64_d)
        id33 = singles.tile([D + 1, D + 1], F32)
        nc.sync.dma_start(out=id33, in_=id33_d)
        id64 = singles.tile([K, K], F32)
        nc.sync.dma_start(out=id64, in_=id64_d)
        eyeneg = singles.tile([K, K], BF16)
        nc.sync.dma_start(out=eyeneg, in_=eyeneg_d)
        iotacol = singles.tile([P, 1], F32)  # = partition index p (0..127)
        nc.sync.dma_start(out=iotacol, in_=iotacol_d)
        epsneg = singles.tile([P, 1], F32)
        nc.vector.memset(epsneg, -EPS)
        epspos = singles.tile([P, 1], F32)
        nc.vector.memset(epspos, EPS)
        margneg = singles.tile([P, 1], F32)
        nc.vector.memset(margneg, -float(INTRA_MARGIN))
        xe = xpool.tile([P, tpc, D + 1], BF16)
        lpm = singles.tile([P, tpc], I16)
        nc.sync.dma_start(out=lpm, in_=lpm_d)
        lpmf = singles.tile([P, tpc], F32)
        nc.sync.dma_start(out=lpmf, in_=lpmf_d)
        iota64f = singles.tile([P, K], F32)
        nc.sync.dma_start(out=iota64f, in_=iota64f_d)

        # ---------- pass 1: segment sums ----------
        psumS = psA.tile([D + 1, K], F32)
        n_chunks1 = math.ceil(tpc / j1)
        t_done = 0
        for c in range(n_chunks1):
            j0 = c * j1
            jn = min(j1, tpc - j0)
            # stream X chunk
            xq = nc.sync if (c % 2 == 0) else nc.scalar
            xq.dma_start(
                out=xe[:, j0 : j0 + jn, :], in_=xe_d[:, j0 : j0 + jn, :]
            )
            h = hpool.tile([P, j1, K], BF16, tag="h")
            if c % 4 == 3:
                half = (jn + 1) // 2
                for s0 in range(0, jn, half):
                    sn = min(half, jn - s0)
                    g0 = j0 + s0
                    tdf = hpool.tile([P, half, K], F32, tag="gtmp")
                    nc.gpsimd.tensor_sub(
                        tdf[:, :sn, :],
                        lpmf[:, g0 : g0 + sn, None].to_broadcast((P, sn, K)),
                        iota64f[:, None, :].to_broadcast((P, sn, K)),
                    )
                    usq = hpool.tile([P, half, K], F32, tag="gtmp")
                    nc.gpsimd.tensor_mul(
                        usq[:, :sn, :], tdf[:, :sn, :], tdf[:, :sn, :]
                    )
                    nc.gpsimd.tensor_scalar_min(
                        usq[:, :sn, :], usq[:, :sn, :], 1.0
                    )
                    nc.gpsimd.tensor_scalar(
                        h[:, s0 : s0 + sn, :], usq[:, :sn, :], -1.0, 1.0,
                        mybir.AluOpType.mult, mybir.AluOpType.add,
                    )
            else:
                nc.vector.tensor_tensor(
                    h[:, :jn, :],
                    lpm[:, j0 : j0 + jn, None].to_broadcast((P, jn, K)),
                    iota64[:, None, :].to_broadcast((P, jn, K)),
                    mybir.AluOpType.is_equal,
                )
            for j in range(jn):
                nc.tensor.matmul(
                    psumS,
                    xe[:, j0 + j, :],
                    h[:, j, :],
                    start=(t_done == 0),
                    stop=(t_done == tpc - 1),
                )
                t_done += 1

        # ---------- AllReduce segment sums ----------
        sg_local = wpool.tile([D + 1, K], F32, tag="sg")
        nc.scalar.copy(out=sg_local, in_=psumS)
        cc_in = dram.tile([D + 1, K], F32)
        cc_out = dram.tile([D + 1, K], F32)
        nc.gpsimd.dma_start(out=cc_in, in_=sg_local)
        nc.gpsimd.collective_compute(
            "AllReduce",
            mybir.AluOpType.add,
            replica_groups=[core_ids],
            ins=[cc_in.opt()],
            outs=[cc_out.opt()],
        )
        sg = wpool.tile([D + 1, K], F32, tag="sg2")
        nc.gpsimd.dma_start(out=sg, in_=cc_out)
        if stage == 1:
            nc.sync.dma_start(out=out_d, in_=sg[0:1, 0:1])

        # ---------- stats: mu, invc, Table, inter, reg ----------
        run_stats = stage >= 2
        # transpose [33, 64] -> [64, 33]
        psW = psS.tile([K, D + 1], F32, tag="small")
        nc.tensor.transpose(psW, sg, id33)
        W = wpool.tile([K, D + 1], F32, tag="w")  # [S_k | c_k]
        nc.scalar.copy(out=W, in_=psW)
        safec = wpool.tile([K, 1], F32, tag="safec")
        nc.vector.tensor_scalar_max(safec, W[:, D : D + 1], 1.0)
        invc = wpool.tile([K, 1], F32, tag="invc")
        nc.vector.reciprocal(invc, safec)
        mu = wpool.tile([K, D], F32, tag="mu")
        nc.vector.tensor_mul(mu, W[:, :D], invc.to_broadcast((K, D)))
        mum = wpool.tile([K, D], F32, tag="mum")  # mu - eps
        nc.vector.tensor_scalar_add(mum, mu, -EPS)
        mup = wpool.tile([K, D], F32, tag="mup")  # mu + eps
        nc.vector.tensor_scalar_add(mup, mu, EPS)
        # q = ||mu||^2, qp = ||mu+eps||^2  (per cluster)
        qsc = wpool.tile([K, D], F32, tag="qsc")
        nc.vector.tensor_mul(qsc, mu, mu)
        q = wpool.tile([K, 1], F32, tag="q")
        nc.vector.tensor_reduce(
            out=q, in_=qsc, axis=mybir.AxisListType.X, op=mybir.AluOpType.add
        )
        qpsc = wpool.tile([K, D], F32, tag="qpsc")
        nc.vector.tensor_mul(qpsc, mup, mup)
        qp = wpool.tile([K, 1], F32, tag="qp")
        nc.vector.tensor_reduce(
            out=qp, in_=qpsc, axis=mybir.AxisListType.X, op=mybir.AluOpType.add
        )
        # Table [64, 33] bf16 = [mu - eps | invc]
        table = singles.tile([P, D + 1], BF16)
        nc.scalar.copy(out=table[:K, :D], in_=mum)
        nc.scalar.copy(out=table[:K, D : D + 1], in_=invc)
        # replicate rows [0,64) -> [64,128) for the B-side matmuls
        nc.sync.dma_start(out=table[K:, :], in_=table[:K, :])

        # inter: pd2[a,b] = qp_a - 2*mup_a.mu_b + q_b
        ab = wpool.tile([K, D + 2], F32, tag="ab")  # [-2*mup | qp | 1]
        nc.scalar.mul(out=ab[:, :D], in_=mup, mul=-2.0)
        nc.scalar.copy(out=ab[:, D : D + 1], in_=qp)
        nc.vector.memset(ab[:, D + 1 : D + 2], 1.0)
        bb = wpool.tile([K, D + 2], F32, tag="bb")  # [mu | 1 | q]
        nc.scalar.copy(out=bb[:, :D], in_=mu)
        nc.vector.memset(bb[:, D : D + 1], 1.0)
        nc.scalar.copy(out=bb[:, D + 1 : D + 2], in_=q)
        psT = psS.tile([D + 2, K], F32, tag="small")
        nc.tensor.transpose(psT, ab, id64)
        atp = wpool.tile([D + 2, K], F32, tag="atp")
        nc.scalar.copy(out=atp, in_=psT)
        psT2 = psS.tile([D + 2, K], F32, tag="small")
        nc.tensor.transpose(psT2, bb, id64)
        btp = wpool.tile([D + 2, K], F32, tag="btp")
        nc.scalar.copy(out=btp, in_=psT2)
        psPD = psS.tile([K, K], F32, tag="small")
        nc.tensor.matmul(psPD, atp, btp)
        pdc = wpool.tile([K, K], F32, tag="pdc")
        nc.vector.tensor_scalar_max(pdc, psPD, 0.0)
        pdist = wpool.tile([K, K], F32, tag="pdist")
        nc.scalar.activation(
            out=pdist, in_=pdc, func=mybir.ActivationFunctionType.Sqrt
        )
        hingeI = wpool.tile([K, K], F32, tag="hingeI")
        nc.scalar.activation(
            out=hingeI, in_=pdist, func=mybir.ActivationFunctionType.Relu,
            bias=float(INTER_MARGIN2), scale=-1.0,
        )
        hm = wpool.tile([K, K], F32, tag="hm")
        nc.vector.tensor_mul(hm, hingeI, eyeneg)
        hm2 = wpool.tile([K, K], F32, tag="hm2")
        nc.vector.tensor_mul(hm2, hm, hm)
        interp = wpool.tile([K, 1], F32, tag="interp")
        nc.vector.tensor_reduce(
            out=interp, in_=hm2, axis=mybir.AxisListType.X, op=mybir.AluOpType.add
        )
        # reg rows: sqrt(qp)
        sqp = wpool.tile([K, 1], F32, tag="sqp")
        nc.scalar.activation(
            out=sqp, in_=qp, func=mybir.ActivationFunctionType.Sqrt
        )
        # partition sums of [interp | sqp] via matmul with ones
        cat2 = wpool.tile([K, 2], F32, tag="cat2")
        nc.scalar.copy(out=cat2[:, 0:1], in_=interp)
        nc.scalar.copy(out=cat2[:, 1:2], in_=sqp)
        ones64 = singles.tile([K, 1], F32)
        nc.vector.memset(ones64, 1.0)
        psIR = psS.tile([1, 2], F32, tag="small")
        nc.tensor.matmul(psIR, ones64, cat2)
        ir = wpool.tile([1, 2], F32, tag="ir")  # [inter_sum, reg_sum]
        nc.scalar.copy(out=ir, in_=psIR)
        if stage == 2:
            nc.sync.dma_start(out=out_d, in_=ir[0:1, 0:1])

        # ---------- pass 2: per-point gather + hinge ----------
        d2all = singles.tile([P, tpc], F32)
        invc_all = singles.tile([P, tpc], F32)
        TPAIR = 30       # pairs per outer chunk (l2/ht granularity)
        JMG = 15         # pairs per PSUM sub-chunk
        nb = tpc - na
        n_outer = math.ceil(na / TPAIR)
        for oc in range(n_outer):
            t0 = oc * TPAIR
            tn = min(TPAIR, na - t0)
            l2 = l2pool.tile([P, TPAIR * P], I16, tag="l2")
            dmaq = nc.sync if (oc % 2 == 0) else nc.scalar
            srcA = ltma_d[t0 * P : (t0 + tn) * P]
            dmaq.dma_start(
                out=l2[:K, : tn * P],
                in_=bass.AP(
                    tensor=srcA.tensor, offset=srcA.offset,
                    ap=[[0, K]] + [[int(s), int(n)] for s, n in srcA.ap],
                ),
            )
            srcB = ltmb_d[t0 * P : (t0 + tn) * P]
            dmaq.dma_start(
                out=l2[K:, : tn * P],
                in_=bass.AP(
                    tensor=srcB.tensor, offset=srcB.offset,
                    ap=[[0, K]] + [[int(s), int(n)] for s, n in srcB.ap],
                ),
            )
            ht = htpool.tile([P, TPAIR * P], BF16, tag="ht")
            nc.vector.tensor_single_scalar(
                ht[:, : tn * P], l2[:, : tn * P], iotacol,
                mybir.AluOpType.is_equal,
            )
            for ic in range(math.ceil(tn / JMG)):
                i0 = ic * JMG
                inn = min(JMG, tn - i0)
                a0 = t0 + i0                      # first A-tile index
                bn = max(0, min(inn, nb - a0))    # B-tiles that exist
                psmgA = psMg.tile([P, JMG, D + 1], F32, tag="psmg")
                for i in range(inn):
                    nc.tensor.matmul(
                        psmgA[:, i, :],
                        ht[:K, (i0 + i) * P : (i0 + i + 1) * P],
                        table[:K, :],
                    )
                dfA = hpool.tile([P, JMG, D], BF16, tag="df")
                nc.vector.tensor_sub(
                    dfA[:, :inn, :],
                    xe[:, a0 : a0 + inn, :D],
                    psmgA[:, :inn, :D],
                )
                nc.scalar.copy(
                    out=invc_all[:, a0 : a0 + inn], in_=psmgA[:, :inn, D]
                )
                sqA = hpool.tile([P, JMG, D], BF16, tag="sq")
                nc.scalar.activation(
                    out=sqA[:, :inn, :], in_=dfA[:, :inn, :],
                    func=mybir.ActivationFunctionType.Square,
                )
                nc.vector.tensor_reduce(
                    out=d2all[:, a0 : a0 + inn], in_=sqA[:, :inn, :],
                    axis=mybir.AxisListType.X, op=mybir.AluOpType.add,
                )
                if bn > 0:
                    b0 = na + a0                  # first B-tile index
                    psmgB = psMg.tile([P, JMG, D + 1], F32, tag="psmg")
                    for i in range(bn):
                        nc.tensor.matmul(
                            psmgB[:, i, :],
                            ht[K:, (i0 + i) * P : (i0 + i + 1) * P],
                            table[K:, :],
                        )
                    dfB = hpool.tile([P, JMG, D], BF16, tag="df")
                    nc.vector.tensor_sub(
                        dfB[:, :bn, :],
                        xe[:, b0 : b0 + bn, :D],
                        psmgB[:, :bn, :D],
                    )
                    nc.scalar.copy(
                        out=invc_all[:, b0 : b0 + bn], in_=psmgB[:, :bn, D]
                    )
                    sqB = hpool.tile([P, JMG, D], BF16, tag="sq")
                    nc.scalar.activation(
                        out=sqB[:, :bn, :], in_=dfB[:, :bn, :],
                        func=mybir.ActivationFunctionType.Square,
                    )
                    nc.vector.tensor_reduce(
                        out=d2all[:, b0 : b0 + bn], in_=sqB[:, :bn, :],
                        axis=mybir.AxisListType.X, op=mybir.AluOpType.add,
                    )

        # ---------- finals ----------
        dist = singles.tile([P, tpc], F32)
        nc.scalar.activation(
            out=dist, in_=d2all, func=mybir.ActivationFunctionType.Sqrt
        )
        nc.scalar.activation(
            out=dist, in_=dist, func=mybir.ActivationFunctionType.Relu,
            bias=margneg,
        )
        nc.vector.tensor_mul(d2all, dist, dist)
        nc.vector.tensor_mul(d2all, d2all, invc_all)
        rowsum = singles.tile([P, 1], F32)
        nc.vector.tensor_reduce(
            out=rowsum, in_=d2all, axis=mybir.AxisListType.X,
            op=mybir.AluOpType.add,
        )
        ones128 = singles.tile([P, 1], F32)
        nc.vector.memset(ones128, 1.0)
        psL = psS.tile([1, 1], F32, tag="small")
        nc.tensor.matmul(psL, rowsum, ones128)
        tot = wpool.tile([1, 3], F32, tag="tot")
        nc.scalar.copy(out=tot[:, 0:1], in_=psL)
        nc.scalar.copy(out=tot[:, 1:3], in_=ir)
        nc.sync.dma_start(out=out_d, in_=tot[0:1, :])

    nc.compile()
    return nc


_NC_CACHE = {}


def _get_program(tpc):
    if tpc not in _NC_CACHE:
        _NC_CACHE[tpc] = build_program(tpc)
    return _NC_CACHE[tpc]


def kernel(features, labels, num_clusters):
    features = np.asarray(features)
    labels = np.asarray(labels)
    n_total = features.shape[0]
    n_core = n_total // N_CORES
    tpc = math.ceil(n_core / P)
    nc = _get_program(tpc)
    in_maps = _host_prep(features, labels, tpc)
    res = run_bass_kernel_spmd(nc, in_maps, list(range(N_CORES)))
    intra_sum = sum(float(res.results[c]["out"][0]) for c in range(N_CORES))
    inter_sum = float(res.results[0]["out"][1])
    reg_sum = float(res.results[0]["out"][2])
    total = (
        intra_sum / K
        + inter_sum / (K * (K - 1))
        + 0.001 * reg_sum / K
    )
    return np.float32(total)



# revision 28
# speedup vs baseline: 1.0524x; 1.0524x over previous
# kernel.py — DiscriminativeLoss on 8 TRN2 NeuronCores (Bass/Tile, SPMD).
#
# Math (matches reference):
#   counts_k = #{i: l_i = k};  S_k = sum_{i in k} x_i;  mu_k = S_k / max(c_k, 1)
#   intra = (1/K) * sum_i invc_{l_i} * relu(||x_i - mu_{l_i} + eps|| - 1.5)^2
#   inter = sum_{a != b} relu(1 - ||(mu_a + eps) - mu_b||)^2 / (K*(K-1))
#   reg   = (1/K) * sum_k ||mu_k + eps||
#   total = intra + inter + 0.001 * reg
#
# V2 design (engine-balanced, cost-model driven):
#   pass 1: one-hot H2 [P, K, jn] via DVE/Pool tensor_tensor is_equal (2x
#     mode: all operands 2-byte packed SBUF); PE matmul lhsT=Xe [128,33],
#     rhs=H2[:, :, j] accumulates S^T = [S | counts] in PSUM [33, 64].
#   AllReduce [33, 64]; stats (mu, 1/c, sqrt(1/c), inter/reg losses) on
#     Act/Pool/PE only, keeping DVE free.
#   pass 2: paired transposed one-hot ht [128, 128] per tile-pair (A on
#     partitions 0-63, B on 64-127) built from broadcast-DMA'd labels via
#     DVE tensor_single_scalar is_equal (4x mode). Per tile, TWO accumulating
#     matmuls produce diff = x - (mu - eps) directly in PSUM:
#       psD  = ht_half^T @ [eps - mu | sqrt(1/c) - 1]   (gather, negated)
#       psD += I_128    @ Xe_tile                        ([x | 1])
#     -> psD = [x - mu + eps | sqrt(1/c)].
#   Act Square psD -> sq bf16 (col 32 squares to 1/c); DVE/Pool tensor_reduce
#     over D -> d2; DVE copies col 32 -> invc_all.
#   finals: dist=sqrt(d2), h=relu(dist-1.5) on Act; intra partial
#     sum_i h^2 * invc via DVE mults + tensor_tensor_reduce + PE ones-matmul.
import math
import numpy as np
from contextlib import ExitStack

import concourse.bass as bass
import concourse.bacc as bacc
import concourse.tile as tile
import concourse.mybir as mybir
from concourse.bass_utils import run_bass_kernel_spmd

F32 = mybir.dt.float32
BF16 = mybir.dt.bfloat16
I16 = mybir.dt.int16

N_CORES = 8
K = 64
D = 32
P = 128
EPS = 1e-8
PAD_LABEL = 999  # never matches any one-hot column

INTRA_MARGIN = 1.5
INTER_MARGIN2 = 1.0  # 2 * 0.5

J1 = 20      # pass-1 chunk width (tiles)
TPAIR = 28   # tile-pairs per ht chunk (4 gather groups of 7 pairs)
JMG = 14     # tiles per PSUM gather group (7 pairs)


def _host_prep(features, labels, tpc):
    """Shard + relayout on host. Returns per-core input dicts."""
    n_total = features.shape[0]
    n_core = n_total // N_CORES
    n_pad = P * tpc
    npair = (tpc + 1) // 2
    import ml_dtypes

    in_maps = []
    for c in range(N_CORES):
        f = np.asarray(features[c * n_core : (c + 1) * n_core], dtype=np.float32)
        l = np.asarray(labels[c * n_core : (c + 1) * n_core], dtype=np.int64)
        if n_pad > n_core:
            f = np.concatenate([f, np.zeros((n_pad - n_core, D), np.float32)], axis=0)
            l = np.concatenate([l, np.full((n_pad - n_core,), PAD_LABEL, np.int64)])
        # Xe: [P, tpc, 33] bf16, col 32 = 1/256 (exact in bf16; keeps the
        # sqrt(1/c) gather free of bf16 cancellation); point i = (i%P, i//P)
        xe = np.full((n_pad, D + 1), 1.0 / 256.0, np.float32)
        xe[:, :D] = f
        xe = xe.reshape(P, tpc, D + 1).astype(ml_dtypes.bfloat16)
        lpj = l.reshape(P, tpc)  # [point-in-tile, tile]
        l_pm = lpj.astype(np.int16)
        # tile-major labels for ht: pair jj = (tile 2jj, tile 2jj+1)
        ltm = lpj.T.astype(np.int16)  # [tpc, P]
        l_tma = np.full((npair, P), PAD_LABEL, np.int16)
        l_tmb = np.full((npair, P), PAD_LABEL, np.int16)
        l_tma[:] = ltm[0::2]
        nb = tpc // 2
        l_tmb[:nb] = ltm[1::2]
        # iotarep [P, K, J1] int16: value k at [:, k, :]
        iotarep = np.tile(
            np.arange(K, dtype=np.int16)[None, :, None], (P, 1, J1)
        )
        in_maps.append(
            {
                "xe": np.ascontiguousarray(xe),
                "labels_pm": np.ascontiguousarray(l_pm),
                "labels_tma": np.ascontiguousarray(l_tma).ravel(),
                "labels_tmb": np.ascontiguousarray(l_tmb).ravel(),
                "iotarep": np.ascontiguousarray(iotarep),
                "iotacol2": np.concatenate(
                    [np.arange(K), np.arange(K)]
                ).astype(np.float32).reshape(P, 1),
                "id128": np.eye(P, dtype=ml_dtypes.bfloat16),
                "id33": np.eye(D + 1, dtype=np.float32),
                "id64": np.eye(K, dtype=np.float32),
                "eyeneg": (1.0 - np.eye(K, dtype=np.float32)).astype(
                    ml_dtypes.bfloat16
                ),
            }
        )
    return in_maps


def build_program(tpc, h2_pool_every=4, red_pool_every=3, stage=4):
    """Build the SPMD Bass program. tpc = tiles per core."""
    nc = bacc.Bacc(
        "TRN2", target_bir_lowering=False, debug=False, num_devices=N_CORES
    )
    core_ids = list(range(N_CORES))
    npair = (tpc + 1) // 2

    xe_d = nc.dram_tensor("xe", [P, tpc, D + 1], BF16, kind="ExternalInput").ap()
    lpm_d = nc.dram_tensor("labels_pm", [P, tpc], I16, kind="ExternalInput").ap()
    ltma_d = nc.dram_tensor("labels_tma", [npair * P], I16, kind="ExternalInput").ap()
    ltmb_d = nc.dram_tensor("labels_tmb", [npair * P], I16, kind="ExternalInput").ap()
    iotarep_d = nc.dram_tensor("iotarep", [P, K, J1], I16, kind="ExternalInput").ap()
    iotacol2_d = nc.dram_tensor("iotacol2", [P, 1], F32, kind="ExternalInput").ap()
    id128_d = nc.dram_tensor("id128", [P, P], BF16, kind="ExternalInput").ap()
    id33_d = nc.dram_tensor("id33", [D + 1, D + 1], F32, kind="ExternalInput").ap()
    id64_d = nc.dram_tensor("id64", [K, K], F32, kind="ExternalInput").ap()
    eyeneg_d = nc.dram_tensor("eyeneg", [K, K], BF16, kind="ExternalInput").ap()
    out_d = nc.dram_tensor("out", [3], F32, kind="ExternalOutput").ap()

    n_chunks1 = math.ceil(tpc / J1)
    n_oc = math.ceil(npair / TPAIR)

    with tile.TileContext(nc, num_cores=N_CORES) as tc, ExitStack() as ctx:
        singles = ctx.enter_context(tc.tile_pool(name="singles", bufs=1))
        xpool = ctx.enter_context(tc.tile_pool(name="xpool", bufs=1))
        h2pool = ctx.enter_context(tc.tile_pool(name="h2pool", bufs=3))
        l2pool = ctx.enter_context(tc.tile_pool(name="l2pool", bufs=4))
        htpool = ctx.enter_context(tc.tile_pool(name="htpool", bufs=4))
        sqpool = ctx.enter_context(tc.tile_pool(name="sqpool", bufs=4))
        wpool = ctx.enter_context(tc.tile_pool(name="wpool", bufs=2))
        psA = ctx.enter_context(tc.tile_pool(name="psA", bufs=1, space="PSUM"))
        psMg = ctx.enter_context(tc.tile_pool(name="psMg", bufs=4, space="PSUM"))
        psS = ctx.enter_context(tc.tile_pool(name="psS", bufs=2, space="PSUM"))
        dram = ctx.enter_context(tc.tile_pool(name="dram", bufs=2, space="DRAM"))

        # ---------- constants ----------
        lpm = singles.tile([P, tpc], I16)
        nc.sync.dma_start(out=lpm, in_=lpm_d)
        iotarep = singles.tile([P, K, J1], I16)
        nc.sync.dma_start(out=iotarep, in_=iotarep_d)
        iotacol2 = singles.tile([P, 1], F32)
        nc.sync.dma_start(out=iotacol2, in_=iotacol2_d)
        id128 = singles.tile([P, P], BF16)
        nc.sync.dma_start(out=id128, in_=id128_d)
        id33 = singles.tile([D + 1, D + 1], F32)
        nc.sync.dma_start(out=id33, in_=id33_d)
        id64 = singles.tile([K, K], F32)
        nc.sync.dma_start(out=id64, in_=id64_d)
        eyeneg = singles.tile([K, K], BF16)
        nc.sync.dma_start(out=eyeneg, in_=eyeneg_d)
        xe = xpool.tile([P, tpc, D + 1], BF16)
        d2all = singles.tile([P, tpc], F32)
        invc_all = singles.tile([P, tpc], BF16)

        # l2 broadcast DMA helper (labels replicated across 64 partitions)
        def issue_l2(oc):
            t0 = oc * TPAIR
            tn = min(TPAIR, npair - t0)
            l2 = l2pool.tile([P, TPAIR * P], I16, tag="l2")
            srcA = ltma_d[t0 * P : (t0 + tn) * P]
            nc.sync.dma_start(
                out=l2[:K, : tn * P],
                in_=bass.AP(
                    tensor=srcA.tensor, offset=srcA.offset,
                    ap=[[0, K]] + [[int(s), int(n)] for s, n in srcA.ap],
                ),
            )
            srcB = ltmb_d[t0 * P : (t0 + tn) * P]
            nc.sync.dma_start(
                out=l2[K:, : tn * P],
                in_=bass.AP(
                    tensor=srcB.tensor, offset=srcB.offset,
                    ap=[[0, K]] + [[int(s), int(n)] for s, n in srcB.ap],
                ),
            )
            return l2, tn

        # ---------- pass 1: segment sums (+ l2 prefetch interleaved) ----------
        psumS = psA.tile([D + 1, K], F32)
        l2_tiles = []
        lc = 0
        t_done = 0
        for c in range(n_chunks1):
            j0 = c * J1
            jn = min(J1, tpc - j0)
            nc.sync.dma_start(
                out=xe[:, j0 : j0 + jn, :], in_=xe_d[:, j0 : j0 + jn, :]
            )
            if c % 3 == 2 and lc < n_oc:
                l2_tiles.append(issue_l2(lc))
                lc += 1
            h2 = h2pool.tile([P, K, J1], BF16, tag="h2")
            nc.vector.tensor_tensor(
                h2[:, :, :jn],
                lpm[:, None, j0 : j0 + jn].to_broadcast((P, K, jn)),
                iotarep[:, :, :jn],
                mybir.AluOpType.is_equal,
            )
            for j in range(jn):
                nc.tensor.matmul(
                    psumS,
                    xe[:, j0 + j, :],
                    h2[:, :, j],
                    start=(t_done == 0),
                    stop=(t_done == tpc - 1),
                )
                t_done += 1
        while lc < n_oc:
            l2_tiles.append(issue_l2(lc))
            lc += 1

        # ---------- ht builds (no AR dependency) ----------
        ht_tiles = []
        for oc in range(n_oc):
            l2, tn = l2_tiles[oc]
            ht = htpool.tile([P, TPAIR * P], BF16, tag="ht")
            nc.vector.tensor_single_scalar(
                ht[:, : tn * P], l2[:, : tn * P], iotacol2,
                mybir.AluOpType.is_equal,
            )
            ht_tiles.append(ht)

        # ---------- AllReduce segment sums ----------
        sg_local = wpool.tile([D + 1, K], F32, tag="sg")
        nc.scalar.copy(out=sg_local, in_=psumS)
        cc_in = dram.tile([D + 1, K], F32)
        cc_out = dram.tile([D + 1, K], F32)
        nc.gpsimd.dma_start(out=cc_in, in_=sg_local)
        nc.gpsimd.collective_compute(
            "AllReduce",
            mybir.AluOpType.add,
            replica_groups=[core_ids],
            ins=[cc_in.opt()],
            outs=[cc_out.opt()],
        )
        sg = wpool.tile([D + 1, K], F32, tag="sg2")
        nc.gpsimd.dma_start(out=sg, in_=cc_out)

        # ---------- stats (Act/Pool/PE only; DVE stays on one-hot work) ----
        psW = psS.tile([K, D + 1], F32, tag="small")
        nc.tensor.transpose(psW, sg, id33)
        W = wpool.tile([K, D + 1], F32, tag="w")  # [S_k | c_k]
        nc.scalar.copy(out=W, in_=psW)
        safec = wpool.tile([K, 1], F32, tag="safec")
        nc.gpsimd.tensor_scalar(
            safec, W[:, D : D + 1], 256.0, 1.0,
            mybir.AluOpType.mult, mybir.AluOpType.max,
        )
        invc = wpool.tile([K, 1], F32, tag="invc")
        nc.vector.reciprocal(invc, safec)
        svp = wpool.tile([K, 1], F32, tag="svp")  # sqrt(1/c)
        nc.scalar.activation(
            out=svp, in_=invc, func=mybir.ActivationFunctionType.Sqrt
        )
        mu = wpool.tile([K, D], F32, tag="mu")
        nc.gpsimd.tensor_mul(mu, W[:, :D], invc.to_broadcast((K, D)))
        # table2 [128, 33] bf16 = [eps - mu | sqrt(1/c) - 1], rows replicated
        table2 = singles.tile([P, D + 1], BF16)
        nc.scalar.activation(
            out=table2[:K, :D], in_=mu,
            func=mybir.ActivationFunctionType.Copy, bias=EPS, scale=-1.0,
        )
        nc.scalar.activation(
            out=table2[:K, D : D + 1], in_=svp,
            func=mybir.ActivationFunctionType.Copy, bias=-1.0 / 256.0,
        )
        nc.sync.dma_start(out=table2[K:, :], in_=table2[:K, :])

        # ---------- inter + reg losses (Act/Pool/PE) ----------
        mup = wpool.tile([K, D], F32, tag="mup")  # mu + eps
        nc.scalar.activation(
            out=mup, in_=mu, func=mybir.ActivationFunctionType.Copy, bias=EPS
        )
        qsc = wpool.tile([K, D], F32, tag="qsc")
        nc.gpsimd.tensor_mul(qsc, mu, mu)
        q = wpool.tile([K, 1], F32, tag="q")  # ||mu||^2
        nc.vector.tensor_reduce(
            out=q, in_=qsc, axis=mybir.AxisListType.X, op=mybir.AluOpType.add
        )
        qpsc = wpool.tile([K, D], F32, tag="qpsc")
        nc.gpsimd.tensor_mul(qpsc, mup, mup)
        qp = wpool.tile([K, 1], F32, tag="qp")  # ||mu + eps||^2
        nc.vector.tensor_reduce(
            out=qp, in_=qpsc, axis=mybir.AxisListType.X, op=mybir.AluOpType.add
        )
        # pd2[a,b] = qp_a - 2*mup_a.mu_b + q_b via [ -2*mup | qp | 1 ] x [ mu | 1 | q ]
        ab = wpool.tile([K, D + 2], F32, tag="ab")
        nc.scalar.mul(out=ab[:, :D], in_=mup, mul=-2.0)
        nc.scalar.copy(out=ab[:, D : D + 1], in_=qp)
        nc.gpsimd.memset(ab[:, D + 1 : D + 2], 1.0)
        bb = wpool.tile([K, D + 2], F32, tag="bb")
        nc.scalar.copy(out=bb[:, :D], in_=mu)
        nc.gpsimd.memset(bb[:, D : D + 1], 1.0)
        nc.scalar.copy(out=bb[:, D + 1 : D + 2], in_=q)
        psT = psS.tile([D + 2, K], F32, tag="small")
        nc.tensor.transpose(psT, ab, id64)
        atp = wpool.tile([D + 2, K], F32, tag="atp")
        nc.scalar.copy(out=atp, in_=psT)
        psT2 = psS.tile([D + 2, K], F32, tag="small")
        nc.tensor.transpose(psT2, bb, id64)
        btp = wpool.tile([D + 2, K], F32, tag="btp")
        nc.scalar.copy(out=btp, in_=psT2)
        psPD = psS.tile([K, K], F32, tag="small")
        nc.tensor.matmul(psPD, atp, btp)
        pdc = wpool.tile([K, K], F32, tag="pdc")
        nc.vector.tensor_scalar_max(pdc, psPD, 0.0)
        pdist = wpool.tile([K, K], F32, tag="pdist")
        nc.scalar.activation(
            out=pdist, in_=pdc, func=mybir.ActivationFunctionType.Sqrt
        )
        hingeI = wpool.tile([K, K], F32, tag="hingeI")
        nc.scalar.activation(
            out=hingeI, in_=pdist, func=mybir.ActivationFunctionType.Relu,
            bias=float(INTER_MARGIN2), scale=-1.0,
        )
        hm = wpool.tile([K, K], F32, tag="hm")
        nc.gpsimd.tensor_mul(hm, hingeI, eyeneg)
        hm2 = wpool.tile([K, K], F32, tag="hm2")
        nc.gpsimd.tensor_mul(hm2, hm, hm)
        interp = wpool.tile([K, 1], F32, tag="interp")
        nc.vector.tensor_reduce(
            out=interp, in_=hm2, axis=mybir.AxisListType.X,
            op=mybir.AluOpType.add,
        )
        sqp = wpool.tile([K, 1], F32, tag="sqp")  # ||mu + eps||
        nc.scalar.activation(
            out=sqp, in_=qp, func=mybir.ActivationFunctionType.Sqrt
        )
        cat2 = wpool.tile([K, 2], F32, tag="cat2")
        nc.scalar.copy(out=cat2[:, 0:1], in_=interp)
        nc.scalar.copy(out=cat2[:, 1:2], in_=sqp)
        ones64 = singles.tile([K, 1], F32)
        nc.gpsimd.memset(ones64, 1.0)
        psIR = psS.tile([1, 2], F32, tag="small")
        nc.tensor.matmul(psIR, ones64, cat2)
        ir = wpool.tile([1, 2], F32, tag="ir")  # [inter_sum, reg_sum]
        nc.scalar.copy(out=ir, in_=psIR)

        # ---------- pass 2: gather + diff in PSUM, square, fold-reduce ------
        for oc in range(n_oc):
            ht = ht_tiles[oc]
            pc = min(TPAIR, npair - oc * TPAIR)
            cbase = oc * TPAIR * 2        # first global tile of this chunk
            ctn = min(pc * 2, tpc - cbase)  # tiles in this chunk
            # one sq tile per ht chunk (up to 56 tiles), 4 PSUM groups
            sq = sqpool.tile([P, TPAIR * 2, D + 1], BF16, tag="sq")
            for g in range(math.ceil(pc / (JMG // 2))):
                p0 = g * (JMG // 2)
                pn = min(JMG // 2, pc - p0)
                jbase = (oc * TPAIR + p0) * 2  # first global tile of group
                nt = min(pn * 2, tpc - jbase)
                psD = psMg.tile([P, JMG, D + 1], F32, tag="psd")
                for lp in range(pn):
                    for half in range(2):
                        t = lp * 2 + half
                        if t >= nt:
                            break
                        j = jbase + t
                        colp = p0 + lp
                        nc.tensor.matmul(
                            psD[:, t, :],
                            ht[64 * half : 64 * (half + 1),
                               colp * P : (colp + 1) * P],
                            table2[64 * half : 64 * (half + 1), :],
                            start=True, stop=False,
                        )
                        nc.tensor.matmul(
                            psD[:, t, :], id128, xe[:, j, :],
                            start=False, stop=True,
                        )
                t0 = p0 * 2
                nc.scalar.activation(
                    out=sq[:, t0 : t0 + nt, :], in_=psD[:, :nt, :],
                    func=mybir.ActivationFunctionType.Square,
                )
            # bf16 fold-tree reduce over D (2x DVE mode), then f32 finish
            with nc.allow_low_precision(reason="bf16 partial sums of d2"):
                f1 = sqpool.tile([P, TPAIR * 2, 16], BF16, tag="f1")
                nc.vector.tensor_add(
                    f1[:, :ctn, :], sq[:, :ctn, 0:16], sq[:, :ctn, 16:32]
                )
                f2 = sqpool.tile([P, TPAIR * 2, 8], BF16, tag="f2")
                nc.vector.tensor_add(
                    f2[:, :ctn, :], f1[:, :ctn, 0:8], f1[:, :ctn, 8:16]
                )
                f3 = sqpool.tile([P, TPAIR * 2, 4], BF16, tag="f3")
                nc.vector.tensor_add(
                    f3[:, :ctn, :], f2[:, :ctn, 0:4], f2[:, :ctn, 4:8]
                )
                f4 = sqpool.tile([P, TPAIR * 2, 2], BF16, tag="f4")
                nc.vector.tensor_add(
                    f4[:, :ctn, :], f3[:, :ctn, 0:2], f3[:, :ctn, 2:4]
                )
            nc.vector.tensor_tensor(
                d2all[:, cbase : cbase + ctn],
                f4[:, :ctn, 0], f4[:, :ctn, 1], mybir.AluOpType.add,
            )
            nc.vector.tensor_scalar_add(
                invc_all[:, cbase : cbase + ctn], sq[:, :ctn, D], 0.0
            )

        # ---------- finals ----------
        nc.scalar.activation(
            out=d2all, in_=d2all, func=mybir.ActivationFunctionType.Sqrt
        )
        margneg = singles.tile([P, 1], F32)
        nc.gpsimd.memset(margneg, -float(INTRA_MARGIN))
        nc.scalar.activation(
            out=d2all, in_=d2all, func=mybir.ActivationFunctionType.Relu,
            bias=margneg,
        )
        hh = singles.tile([P, tpc], F32)
        nc.gpsimd.tensor_mul(hh, d2all, d2all)
        hhw = singles.tile([P, tpc], F32)
        nc.gpsimd.tensor_mul(hhw, hh, invc_all)
        rowsum = singles.tile([P, 1], F32)
        nc.vector.tensor_reduce(
            out=rowsum, in_=hhw, axis=mybir.AxisListType.X,
            op=mybir.AluOpType.add,
        )
        ones128 = singles.tile([P, 1], F32)
        nc.gpsimd.memset(ones128, 1.0)
        psL = psS.tile([1, 1], F32, tag="small")
        nc.tensor.matmul(psL, rowsum, ones128)
        tot = wpool.tile([1, 3], F32, tag="tot")
        nc.scalar.copy(out=tot[:, 0:1], in_=psL)
        nc.scalar.copy(out=tot[:, 1:3], in_=ir)
        nc.sync.dma_start(out=out_d, in_=tot[0:1, :])

    nc.compile()
    return nc


_NC_CACHE = {}


def _get_program(tpc):
    if tpc not in _NC_CACHE:
        _NC_CACHE[tpc] = build_program(tpc)
    return _NC_CACHE[tpc]


def kernel(features, labels, num_clusters):
    features = np.asarray(features)
    labels = np.asarray(labels)
    n_total = features.shape[0]
    n_core = n_total // N_CORES
    tpc = math.ceil(n_core / P)
    nc = _get_program(tpc)
    in_maps = _host_prep(features, labels, tpc)
    res = run_bass_kernel_spmd(nc, in_maps, list(range(N_CORES)))
    intra_sum = sum(float(res.results[c]["out"][0]) for c in range(N_CORES))
    inter_sum = float(res.results[0]["out"][1])
    reg_sum = float(res.results[0]["out"][2])
    total = (
        intra_sum / K
        + inter_sum / (K * (K - 1))
        + 0.001 * reg_sum / K
    )
    return np.float32(total)


# revision 31
# speedup vs baseline: 1.2717x; 1.2084x over previous
# kernel.py — DiscriminativeLoss on 8 TRN2 NeuronCores (Bass/Tile, SPMD).
#
# Math (matches reference):
#   counts_k = #{i: l_i = k};  S_k = sum_{i in k} x_i;  mu_k = S_k / max(c_k, 1)
#   intra = (1/K) * sum_i invc_{l_i} * relu(||x_i - mu_{l_i} + eps|| - 1.5)^2
#   inter = sum_{a != b} relu(1 - ||(mu_a + eps) - mu_b||)^2 / (K*(K-1))
#   reg   = (1/K) * sum_k ||mu_k + eps||
#   total = intra + inter + 0.001 * reg
#
# V2 design (engine-balanced, cost-model driven):
#   pass 1: one-hot H2 [P, K, jn] via DVE/Pool tensor_tensor is_equal (2x
#     mode: all operands 2-byte packed SBUF); PE matmul lhsT=Xe [128,33],
#     rhs=H2[:, :, j] accumulates S^T = [S | counts] in PSUM [33, 64].
#   AllReduce [33, 64]; stats (mu, 1/c, sqrt(1/c), inter/reg losses) on
#     Act/Pool/PE only, keeping DVE free.
#   pass 2: paired transposed one-hot ht [128, 128] per tile-pair (A on
#     partitions 0-63, B on 64-127) built from broadcast-DMA'd labels via
#     DVE tensor_single_scalar is_equal (4x mode). Per tile, TWO accumulating
#     matmuls produce diff = x - (mu - eps) directly in PSUM:
#       psD  = ht_half^T @ [eps - mu | sqrt(1/c) - 1]   (gather, negated)
#       psD += I_128    @ Xe_tile                        ([x | 1])
#     -> psD = [x - mu + eps | sqrt(1/c)].
#   Act Square psD -> sq bf16 (col 32 squares to 1/c); DVE/Pool tensor_reduce
#     over D -> d2; DVE copies col 32 -> invc_all.
#   finals: dist=sqrt(d2), h=relu(dist-1.5) on Act; intra partial
#     sum_i h^2 * invc via DVE mults + tensor_tensor_reduce + PE ones-matmul.
import math
import numpy as np
from contextlib import ExitStack

import concourse.bass as bass
import concourse.bacc as bacc
import concourse.tile as tile
import concourse.mybir as mybir
from concourse.bass_utils import run_bass_kernel_spmd

F32 = mybir.dt.float32
BF16 = mybir.dt.bfloat16
I16 = mybir.dt.int16

N_CORES = 8
K = 64
D = 32
P = 128
EPS = 1e-8
PAD_LABEL = 999  # never matches any one-hot column

INTRA_MARGIN = 1.5
INTER_MARGIN2 = 1.0  # 2 * 0.5

J1 = 20      # pass-1 chunk width (tiles)
TPAIR = 28   # tile-pairs per ht chunk (4 gather groups of 7 pairs)
JMG = 14     # tiles per PSUM gather group (7 pairs)


def _host_prep(features, labels, tpc):
    """Shard + relayout on host. Returns per-core input dicts."""
    n_total = features.shape[0]
    n_core = n_total // N_CORES
    n_pad = P * tpc
    npair = (tpc + 1) // 2
    import ml_dtypes

    in_maps = []
    for c in range(N_CORES):
        f = np.asarray(features[c * n_core : (c + 1) * n_core], dtype=np.float32)
        l = np.asarray(labels[c * n_core : (c + 1) * n_core], dtype=np.int64)
        if n_pad > n_core:
            f = np.concatenate([f, np.zeros((n_pad - n_core, D), np.float32)], axis=0)
            l = np.concatenate([l, np.full((n_pad - n_core,), PAD_LABEL, np.int64)])
        # Xe: [P, tpc, 33] bf16, col 32 = 1/256 (exact in bf16; keeps the
        # sqrt(1/c) gather free of bf16 cancellation); point i = (i%P, i//P)
        xe = np.full((n_pad, D + 1), 1.0 / 256.0, np.float32)
        xe[:, :D] = f
        xe = xe.reshape(P, tpc, D + 1).astype(ml_dtypes.bfloat16)
        lpj = l.reshape(P, tpc)  # [point-in-tile, tile]
        l_pm = lpj.astype(np.int16)
        # tile-major labels for ht: pair jj = (tile 2jj, tile 2jj+1)
        ltm = lpj.T.astype(np.int16)  # [tpc, P]
        l_tma = np.full((npair, P), PAD_LABEL, np.int16)
        l_tmb = np.full((npair, P), PAD_LABEL, np.int16)
        l_tma[:] = ltm[0::2]
        nb = tpc // 2
        l_tmb[:nb] = ltm[1::2]
        # iotarep [P, K, J1] int16: value k at [:, k, :]
        iotarep = np.tile(
            np.arange(K, dtype=np.int16)[None, :, None], (P, 1, J1)
        )
        in_maps.append(
            {
                "xe": np.ascontiguousarray(xe),
                "labels_pm": np.ascontiguousarray(l_pm),
                "labels_tma": np.ascontiguousarray(l_tma).ravel(),
                "labels_tmb": np.ascontiguousarray(l_tmb).ravel(),
                "iotarep": np.ascontiguousarray(iotarep),
                "iotacol2": np.concatenate(
                    [np.arange(K), np.arange(K)]
                ).astype(np.float32).reshape(P, 1),
                "id128": np.eye(P, dtype=ml_dtypes.bfloat16),
                "id33": np.eye(D + 1, dtype=np.float32),
                "id64": np.eye(K, dtype=np.float32),
                "eyeneg": (1.0 - np.eye(K, dtype=np.float32)).astype(
                    ml_dtypes.bfloat16
                ),
            }
        )
    return in_maps


def build_program(tpc, h2_pool_every=4, red_pool_every=3, stage=4):
    """Build the SPMD Bass program. tpc = tiles per core."""
    nc = bacc.Bacc(
        "TRN2", target_bir_lowering=False, debug=False, num_devices=N_CORES
    )
    core_ids = list(range(N_CORES))
    npair = (tpc + 1) // 2

    xe_d = nc.dram_tensor("xe", [P, tpc, D + 1], BF16, kind="ExternalInput").ap()
    lpm_d = nc.dram_tensor("labels_pm", [P, tpc], I16, kind="ExternalInput").ap()
    ltma_d = nc.dram_tensor("labels_tma", [npair * P], I16, kind="ExternalInput").ap()
    ltmb_d = nc.dram_tensor("labels_tmb", [npair * P], I16, kind="ExternalInput").ap()
    iotarep_d = nc.dram_tensor("iotarep", [P, K, J1], I16, kind="ExternalInput").ap()
    iotacol2_d = nc.dram_tensor("iotacol2", [P, 1], F32, kind="ExternalInput").ap()
    id128_d = nc.dram_tensor("id128", [P, P], BF16, kind="ExternalInput").ap()
    id33_d = nc.dram_tensor("id33", [D + 1, D + 1], F32, kind="ExternalInput").ap()
    id64_d = nc.dram_tensor("id64", [K, K], F32, kind="ExternalInput").ap()
    eyeneg_d = nc.dram_tensor("eyeneg", [K, K], BF16, kind="ExternalInput").ap()
    out_d = nc.dram_tensor("out", [3], F32, kind="ExternalOutput").ap()

    n_chunks1 = math.ceil(tpc / J1)
    n_oc = math.ceil(npair / TPAIR)

    with tile.TileContext(nc, num_cores=N_CORES) as tc, ExitStack() as ctx:
        singles = ctx.enter_context(tc.tile_pool(name="singles", bufs=1))
        xpool = ctx.enter_context(tc.tile_pool(name="xpool", bufs=1))
        h2pool = ctx.enter_context(tc.tile_pool(name="h2pool", bufs=3))
        l2pool = ctx.enter_context(tc.tile_pool(name="l2pool", bufs=4))
        htpool = ctx.enter_context(tc.tile_pool(name="htpool", bufs=4))
        sqpool = ctx.enter_context(tc.tile_pool(name="sqpool", bufs=4))
        wpool = ctx.enter_context(tc.tile_pool(name="wpool", bufs=2))
        psA = ctx.enter_context(tc.tile_pool(name="psA", bufs=1, space="PSUM"))
        psMg = ctx.enter_context(tc.tile_pool(name="psMg", bufs=4, space="PSUM"))
        psS = ctx.enter_context(tc.tile_pool(name="psS", bufs=2, space="PSUM"))
        dram = ctx.enter_context(tc.tile_pool(name="dram", bufs=2, space="DRAM"))

        # ---------- constants ----------
        lpm = singles.tile([P, tpc], I16)
        nc.sync.dma_start(out=lpm, in_=lpm_d)
        iotarep = singles.tile([P, K, J1], I16)
        nc.sync.dma_start(out=iotarep, in_=iotarep_d)
        iotacol2 = singles.tile([P, 1], F32)
        nc.sync.dma_start(out=iotacol2, in_=iotacol2_d)
        id128 = singles.tile([P, P], BF16)
        nc.sync.dma_start(out=id128, in_=id128_d)
        id33 = singles.tile([D + 1, D + 1], F32)
        nc.sync.dma_start(out=id33, in_=id33_d)
        id64 = singles.tile([K, K], F32)
        nc.sync.dma_start(out=id64, in_=id64_d)
        eyeneg = singles.tile([K, K], BF16)
        nc.sync.dma_start(out=eyeneg, in_=eyeneg_d)
        xe = xpool.tile([P, tpc, D + 1], BF16)
        d2all = singles.tile([P, tpc], F32)
        invc_all = singles.tile([P, tpc], BF16)

        # l2 broadcast DMA helper (labels replicated across 64 partitions)
        def issue_l2(oc):
            t0 = oc * TPAIR
            tn = min(TPAIR, npair - t0)
            l2 = l2pool.tile([P, TPAIR * P], I16, tag="l2")
            srcA = ltma_d[t0 * P : (t0 + tn) * P]
            nc.sync.dma_start(
                out=l2[:K, : tn * P],
                in_=bass.AP(
                    tensor=srcA.tensor, offset=srcA.offset,
                    ap=[[0, K]] + [[int(s), int(n)] for s, n in srcA.ap],
                ),
            )
            srcB = ltmb_d[t0 * P : (t0 + tn) * P]
            nc.sync.dma_start(
                out=l2[K:, : tn * P],
                in_=bass.AP(
                    tensor=srcB.tensor, offset=srcB.offset,
                    ap=[[0, K]] + [[int(s), int(n)] for s, n in srcB.ap],
                ),
            )
            return l2, tn

        # ---------- pass 1: segment sums ----------
        # xe DMAs issued first so pass-1 is never starved by the (large)
        # l2 broadcast transfers; only a few l2 chunks go early to warm the
        # ht pipeline, the rest are issued after the AllReduce input DMA so
        # the collective is not queued behind them on the DMA engines.
        psumS = psA.tile([D + 1, K], F32)
        l2_tiles = []
        lc = 0
        t_done = 0
        for c in range(n_chunks1):
            j0 = c * J1
            jn = min(J1, tpc - j0)
            nc.sync.dma_start(
                out=xe[:, j0 : j0 + jn, :], in_=xe_d[:, j0 : j0 + jn, :]
            )
            if c >= n_chunks1 - 4 and lc < 4:
                l2_tiles.append(issue_l2(lc))
                lc += 1
            h2 = h2pool.tile([P, K, J1], BF16, tag="h2")
            nc.vector.tensor_tensor(
                h2[:, :, :jn],
                lpm[:, None, j0 : j0 + jn].to_broadcast((P, K, jn)),
                iotarep[:, :, :jn],
                mybir.AluOpType.is_equal,
            )
            for j in range(jn):
                nc.tensor.matmul(
                    psumS,
                    xe[:, j0 + j, :],
                    h2[:, :, j],
                    start=(t_done == 0),
                    stop=(t_done == tpc - 1),
                )
                t_done += 1
        # ---------- AllReduce segment sums ----------
        sg_local = wpool.tile([D + 1, K], F32, tag="sg")
        nc.scalar.copy(out=sg_local, in_=psumS)
        cc_in = dram.tile([D + 1, K], F32)
        cc_out = dram.tile([D + 1, K], F32)
        nc.gpsimd.dma_start(out=cc_in, in_=sg_local)
        nc.gpsimd.collective_compute(
            "AllReduce",
            mybir.AluOpType.add,
            replica_groups=[core_ids],
            ins=[cc_in.opt()],
            outs=[cc_out.opt()],
        )
        sg = wpool.tile([D + 1, K], F32, tag="sg2")
        nc.gpsimd.dma_start(out=sg, in_=cc_out)

        # remaining l2 broadcasts (transfer during the AllReduce window)
        while lc < n_oc:
            l2_tiles.append(issue_l2(lc))
            lc += 1

        # ---------- ht builds (no AR dependency) ----------
        ht_tiles = []
        for oc in range(n_oc):
            l2, tn = l2_tiles[oc]
            ht = htpool.tile([P, TPAIR * P], BF16, tag="ht")
            nc.vector.tensor_single_scalar(
                ht[:, : tn * P], l2[:, : tn * P], iotacol2,
                mybir.AluOpType.is_equal,
            )
            ht_tiles.append(ht)

        # ---------- stats (Act/Pool/PE only; DVE stays on one-hot work) ----
        psW = psS.tile([K, D + 1], F32, tag="small")
        nc.tensor.transpose(psW, sg, id33)
        W = wpool.tile([K, D + 1], F32, tag="w")  # [S_k | c_k]
        nc.scalar.copy(out=W, in_=psW)
        safec = wpool.tile([K, 1], F32, tag="safec")
        nc.gpsimd.tensor_scalar(
            safec, W[:, D : D + 1], 256.0, 1.0,
            mybir.AluOpType.mult, mybir.AluOpType.max,
        )
        invc = wpool.tile([K, 1], F32, tag="invc")
        nc.vector.reciprocal(invc, safec)
        svp = wpool.tile([K, 1], F32, tag="svp")  # sqrt(1/c)
        nc.scalar.activation(
            out=svp, in_=invc, func=mybir.ActivationFunctionType.Sqrt
        )
        mu = wpool.tile([K, D], F32, tag="mu")
        nc.gpsimd.tensor_mul(mu, W[:, :D], invc.to_broadcast((K, D)))
        # table2 [128, 33] bf16 = [eps - mu | sqrt(1/c) - 1], rows replicated
        table2 = singles.tile([P, D + 1], BF16)
        nc.scalar.activation(
            out=table2[:K, :D], in_=mu,
            func=mybir.ActivationFunctionType.Copy, bias=EPS, scale=-1.0,
        )
        nc.scalar.activation(
            out=table2[:K, D : D + 1], in_=svp,
            func=mybir.ActivationFunctionType.Copy, bias=-1.0 / 256.0,
        )
        nc.sync.dma_start(out=table2[K:, :], in_=table2[:K, :])

        # ---------- inter + reg losses (Act/Pool/PE) ----------
        mup = wpool.tile([K, D], F32, tag="mup")  # mu + eps
        nc.scalar.activation(
            out=mup, in_=mu, func=mybir.ActivationFunctionType.Copy, bias=EPS
        )
        qsc = wpool.tile([K, D], F32, tag="qsc")
        nc.gpsimd.tensor_mul(qsc, mu, mu)
        q = wpool.tile([K, 1], F32, tag="q")  # ||mu||^2
        nc.vector.tensor_reduce(
            out=q, in_=qsc, axis=mybir.AxisListType.X, op=mybir.AluOpType.add
        )
        qpsc = wpool.tile([K, D], F32, tag="qpsc")
        nc.gpsimd.tensor_mul(qpsc, mup, mup)
        qp = wpool.tile([K, 1], F32, tag="qp")  # ||mu + eps||^2
        nc.vector.tensor_reduce(
            out=qp, in_=qpsc, axis=mybir.AxisListType.X, op=mybir.AluOpType.add
        )
        # pd2[a,b] = qp_a - 2*mup_a.mu_b + q_b via [ -2*mup | qp | 1 ] x [ mu | 1 | q ]
        ab = wpool.tile([K, D + 2], F32, tag="ab")
        nc.scalar.mul(out=ab[:, :D], in_=mup, mul=-2.0)
        nc.scalar.copy(out=ab[:, D : D + 1], in_=qp)
        nc.gpsimd.memset(ab[:, D + 1 : D + 2], 1.0)
        bb = wpool.tile([K, D + 2], F32, tag="bb")
        nc.scalar.copy(out=bb[:, :D], in_=mu)
        nc.gpsimd.memset(bb[:, D : D + 1], 1.0)
        nc.scalar.copy(out=bb[:, D + 1 : D + 2], in_=q)
        psT = psS.tile([D + 2, K], F32, tag="small")
        nc.tensor.transpose(psT, ab, id64)
        atp = wpool.tile([D + 2, K], F32, tag="atp")
        nc.scalar.copy(out=atp, in_=psT)
        psT2 = psS.tile([D + 2, K], F32, tag="small")
        nc.tensor.transpose(psT2, bb, id64)
        btp = wpool.tile([D + 2, K], F32, tag="btp")
        nc.scalar.copy(out=btp, in_=psT2)
        psPD = psS.tile([K, K], F32, tag="small")
        nc.tensor.matmul(psPD, atp, btp)
        pdc = wpool.tile([K, K], F32, tag="pdc")
        nc.vector.tensor_scalar_max(pdc, psPD, 0.0)
        pdist = wpool.tile([K, K], F32, tag="pdist")
        nc.scalar.activation(
            out=pdist, in_=pdc, func=mybir.ActivationFunctionType.Sqrt
        )
        hingeI = wpool.tile([K, K], F32, tag="hingeI")
        nc.scalar.activation(
            out=hingeI, in_=pdist, func=mybir.ActivationFunctionType.Relu,
            bias=float(INTER_MARGIN2), scale=-1.0,
        )
        hm = wpool.tile([K, K], F32, tag="hm")
        nc.gpsimd.tensor_mul(hm, hingeI, eyeneg)
        hm2 = wpool.tile([K, K], F32, tag="hm2")
        nc.gpsimd.tensor_mul(hm2, hm, hm)
        interp = wpool.tile([K, 1], F32, tag="interp")
        nc.vector.tensor_reduce(
            out=interp, in_=hm2, axis=mybir.AxisListType.X,
            op=mybir.AluOpType.add,
        )
        sqp = wpool.tile([K, 1], F32, tag="sqp")  # ||mu + eps||
        nc.scalar.activation(
            out=sqp, in_=qp, func=mybir.ActivationFunctionType.Sqrt
        )
        cat2 = wpool.tile([K, 2], F32, tag="cat2")
        nc.scalar.copy(out=cat2[:, 0:1], in_=interp)
        nc.scalar.copy(out=cat2[:, 1:2], in_=sqp)
        ones64 = singles.tile([K, 1], F32)
        nc.gpsimd.memset(ones64, 1.0)
        psIR = psS.tile([1, 2], F32, tag="small")
        nc.tensor.matmul(psIR, ones64, cat2)
        ir = wpool.tile([1, 2], F32, tag="ir")  # [inter_sum, reg_sum]
        nc.scalar.copy(out=ir, in_=psIR)

        # ---------- pass 2: gather + diff in PSUM, square, fold-reduce ------
        for oc in range(n_oc):
            ht = ht_tiles[oc]
            pc = min(TPAIR, npair - oc * TPAIR)
            cbase = oc * TPAIR * 2        # first global tile of this chunk
            ctn = min(pc * 2, tpc - cbase)  # tiles in this chunk
            # one sq tile per ht chunk (up to 56 tiles), 4 PSUM groups
            sq = sqpool.tile([P, TPAIR * 2, D + 1], BF16, tag="sq")
            for g in range(math.ceil(pc / (JMG // 2))):
                p0 = g * (JMG // 2)
                pn = min(JMG // 2, pc - p0)
                jbase = (oc * TPAIR + p0) * 2  # first global tile of group
                nt = min(pn * 2, tpc - jbase)
                psD = psMg.tile([P, JMG, D + 1], F32, tag="psd")
                for lp in range(pn):
                    for half in range(2):
                        t = lp * 2 + half
                        if t >= nt:
                            break
                        j = jbase + t
                        colp = p0 + lp
                        nc.tensor.matmul(
                            psD[:, t, :],
                            ht[64 * half : 64 * (half + 1),
                               colp * P : (colp + 1) * P],
                            table2[64 * half : 64 * (half + 1), :],
                            start=True, stop=False,
                        )
                        nc.tensor.matmul(
                            psD[:, t, :], id128, xe[:, j, :],
                            start=False, stop=True,
                        )
                t0 = p0 * 2
                if g % 4 == 3:
                    # every 4th square on DVE to balance Act post-AR load
                    nc.vector.tensor_tensor(
                        sq[:, t0 : t0 + nt, :], psD[:, :nt, :],
                        psD[:, :nt, :], mybir.AluOpType.mult,
                    )
                else:
                    nc.scalar.activation(
                        out=sq[:, t0 : t0 + nt, :], in_=psD[:, :nt, :],
                        func=mybir.ActivationFunctionType.Square,
                    )
            # bf16 fold-tree reduce over D (2x DVE mode), then f32 finish
            with nc.allow_low_precision(reason="bf16 partial sums of d2"):
                f1 = sqpool.tile([P, TPAIR * 2, 16], BF16, tag="f1")
                nc.vector.tensor_add(
                    f1[:, :ctn, :], sq[:, :ctn, 0:16], sq[:, :ctn, 16:32]
                )
                f2 = sqpool.tile([P, TPAIR * 2, 8], BF16, tag="f2")
                nc.vector.tensor_add(
                    f2[:, :ctn, :], f1[:, :ctn, 0:8], f1[:, :ctn, 8:16]
                )
                f3 = sqpool.tile([P, TPAIR * 2, 4], BF16, tag="f3")
                nc.vector.tensor_add(
                    f3[:, :ctn, :], f2[:, :ctn, 0:4], f2[:, :ctn, 4:8]
                )
                f4 = sqpool.tile([P, TPAIR * 2, 2], BF16, tag="f4")
                nc.vector.tensor_add(
                    f4[:, :ctn, :], f3[:, :ctn, 0:2], f3[:, :ctn, 2:4]
                )
            nc.vector.tensor_tensor(
                d2all[:, cbase : cbase + ctn],
                f4[:, :ctn, 0], f4[:, :ctn, 1], mybir.AluOpType.add,
            )
            nc.vector.tensor_scalar_add(
                invc_all[:, cbase : cbase + ctn], sq[:, :ctn, D], 0.0
            )

        # ---------- finals ----------
        nc.scalar.activation(
            out=d2all, in_=d2all, func=mybir.ActivationFunctionType.Sqrt
        )
        margneg = singles.tile([P, 1], F32)
        nc.gpsimd.memset(margneg, -float(INTRA_MARGIN))
        nc.scalar.activation(
            out=d2all, in_=d2all, func=mybir.ActivationFunctionType.Relu,
            bias=margneg,
        )
        hh = singles.tile([P, tpc], F32)
        nc.gpsimd.tensor_mul(hh, d2all, d2all)
        hhw = singles.tile([P, tpc], F32)
        nc.gpsimd.tensor_mul(hhw, hh, invc_all)
        rowsum = singles.tile([P, 1], F32)
        nc.vector.tensor_reduce(
            out=rowsum, in_=hhw, axis=mybir.AxisListType.X,
            op=mybir.AluOpType.add,
        )
        ones128 = singles.tile([P, 1], F32)
        nc.gpsimd.memset(ones128, 1.0)
        psL = psS.tile([1, 1], F32, tag="small")
        nc.tensor.matmul(psL, rowsum, ones128)
        tot = wpool.tile([1, 3], F32, tag="tot")
        nc.scalar.copy(out=tot[:, 0:1], in_=psL)
        nc.scalar.copy(out=tot[:, 1:3], in_=ir)
        nc.sync.dma_start(out=out_d, in_=tot[0:1, :])

    nc.compile()
    return nc


_NC_CACHE = {}


def _get_program(tpc):
    if tpc not in _NC_CACHE:
        _NC_CACHE[tpc] = build_program(tpc)
    return _NC_CACHE[tpc]


def kernel(features, labels, num_clusters):
    features = np.asarray(features)
    labels = np.asarray(labels)
    n_total = features.shape[0]
    n_core = n_total // N_CORES
    tpc = math.ceil(n_core / P)
    nc = _get_program(tpc)
    in_maps = _host_prep(features, labels, tpc)
    res = run_bass_kernel_spmd(nc, in_maps, list(range(N_CORES)))
    intra_sum = sum(float(res.results[c]["out"][0]) for c in range(N_CORES))
    inter_sum = float(res.results[0]["out"][1])
    reg_sum = float(res.results[0]["out"][2])
    total = (
        intra_sum / K
        + inter_sum / (K * (K - 1))
        + 0.001 * reg_sum / K
    )
    return np.float32(total)


# revision 51
# speedup vs baseline: 1.6490x; 1.2967x over previous
# kernel.py — DiscriminativeLoss on 8 TRN2 NeuronCores (Bass/Tile, SPMD).
#
# Math (matches reference):
#   counts_k = #{i: l_i = k};  S_k = sum_{i in k} x_i;  mu_k = S_k / max(c_k, 1)
#   intra = (1/K) * sum_i invc_{l_i} * relu(||x_i - mu_{l_i} + eps|| - 1.5)^2
#   inter = sum_{a != b} relu(1 - ||(mu_a + eps) - mu_b||)^2 / (K*(K-1))
#   reg   = (1/K) * sum_k ||mu_k + eps||
#   total = intra + inter + 0.001 * reg
#
# V2 design (engine-balanced, cost-model driven):
#   pass 1: one-hot H2 [P, K, jn] via DVE/Pool tensor_tensor is_equal (2x
#     mode: all operands 2-byte packed SBUF); PE matmul lhsT=Xe [128,33],
#     rhs=H2[:, :, j] accumulates S^T = [S | counts] in PSUM [33, 64].
#   AllReduce [33, 64]; stats (mu, 1/c, sqrt(1/c), inter/reg losses) on
#     Act/Pool/PE only, keeping DVE free.
#   pass 2: paired transposed one-hot ht [128, 128] per tile-pair (A on
#     partitions 0-63, B on 64-127) built from broadcast-DMA'd labels via
#     DVE tensor_single_scalar is_equal (4x mode). Per tile, TWO accumulating
#     matmuls produce diff = x - (mu - eps) directly in PSUM:
#       psD  = ht_half^T @ [eps - mu | sqrt(1/c) - 1]   (gather, negated)
#       psD += I_128    @ Xe_tile                        ([x | 1])
#     -> psD = [x - mu + eps | sqrt(1/c)].
#   Act Square psD -> sq bf16 (col 32 squares to 1/c); DVE/Pool tensor_reduce
#     over D -> d2; DVE copies col 32 -> invc_all.
#   finals: dist=sqrt(d2), h=relu(dist-1.5) on Act; intra partial
#     sum_i h^2 * invc via DVE mults + tensor_tensor_reduce + PE ones-matmul.
import math
import numpy as np
from contextlib import ExitStack

import concourse.bass as bass
import concourse.bacc as bacc
import concourse.tile as tile
import concourse.mybir as mybir
from concourse.bass_utils import run_bass_kernel_spmd

F32 = mybir.dt.float32
BF16 = mybir.dt.bfloat16
I16 = mybir.dt.int16

N_CORES = 8
K = 64
D = 32
P = 128
EPS = 1e-8
PAD_LABEL = 999  # never matches any one-hot column

INTRA_MARGIN = 1.5
INTER_MARGIN2 = 1.0  # 2 * 0.5

J1 = 20      # pass-1 chunk width (tiles)
TPAIR = 28   # tile-pairs per ht chunk (4 gather groups of 7 pairs)
JMG = 14     # tiles per PSUM gather group (7 pairs)


def _host_prep(features, labels, tpc):
    """Shard + relayout on host. Returns per-core input dicts."""
    n_total = features.shape[0]
    n_core = n_total // N_CORES
    n_pad = P * tpc
    npair = (tpc + 1) // 2
    import ml_dtypes

    in_maps = []
    for c in range(N_CORES):
        f = np.asarray(features[c * n_core : (c + 1) * n_core], dtype=np.float32)
        l = np.asarray(labels[c * n_core : (c + 1) * n_core], dtype=np.int64)
        if n_pad > n_core:
            f = np.concatenate([f, np.zeros((n_pad - n_core, D), np.float32)], axis=0)
            l = np.concatenate([l, np.full((n_pad - n_core,), PAD_LABEL, np.int64)])
        # Xe: [P, tpc, 33] bf16, col 32 = 1/256 (exact in bf16; keeps the
        # sqrt(1/c) gather free of bf16 cancellation); point i = (i%P, i//P)
        xe = np.full((n_pad, D + 1), 1.0 / 256.0, np.float32)
        xe[:, :D] = f
        xe = xe.reshape(P, tpc, D + 1).astype(ml_dtypes.bfloat16)
        lpj = l.reshape(P, tpc)  # [point-in-tile, tile]
        l_pm = lpj.astype(np.int16)
        # paired label broadcast for ht, fully materialized on host:
        # [128, npair*128] int16, rows 0-63 = labels of tile 2jj, rows
        # 64-127 = labels of tile 2jj+1 (one contiguous DMA per chunk)
        ltm = lpj.T.astype(np.int16)  # [tpc, P]
        l_tma = np.full((npair, P), PAD_LABEL, np.int16)
        l_tmb = np.full((npair, P), PAD_LABEL, np.int16)
        l_tma[:] = ltm[0::2]
        nb = tpc // 2
        l_tmb[:nb] = ltm[1::2]
        l2full = np.empty((P, npair * P), np.int16)
        l2full[:K] = np.broadcast_to(
            l_tma.reshape(1, npair * P), (K, npair * P)
        )
        l2full[K:] = np.broadcast_to(
            l_tmb.reshape(1, npair * P), (K, npair * P)
        )
        # iotarep [P, K, J1] int16: value k at [:, k, :]
        iotarep = np.tile(
            np.arange(K, dtype=np.int16)[None, :, None], (P, 1, J1)
        )
        in_maps.append(
            {
                "xe": np.ascontiguousarray(xe),
                "labels_pm": np.ascontiguousarray(l_pm),
                "l2full": l2full,
                "iotarep": np.ascontiguousarray(iotarep),
                "iotacol2": np.concatenate(
                    [np.arange(K), np.arange(K)]
                ).astype(np.float32).reshape(P, 1),
                "id128": np.eye(P, dtype=ml_dtypes.bfloat16),
                "id33": np.eye(D + 1, dtype=np.float32),
                "id64": np.eye(K, dtype=np.float32),
                "eyeneg": (1.0 - np.eye(K, dtype=np.float32)).astype(
                    ml_dtypes.bfloat16
                ),
            }
        )
    return in_maps


def build_program(tpc, h2_pool_every=4, red_pool_every=3, stage=4):
    """Build the SPMD Bass program. tpc = tiles per core."""
    nc = bacc.Bacc(
        "TRN2", target_bir_lowering=False, debug=False, num_devices=N_CORES
    )
    core_ids = list(range(N_CORES))
    npair = (tpc + 1) // 2

    xe_d = nc.dram_tensor("xe", [P, tpc, D + 1], BF16, kind="ExternalInput").ap()
    lpm_d = nc.dram_tensor("labels_pm", [P, tpc], I16, kind="ExternalInput").ap()
    l2f_d = nc.dram_tensor("l2full", [P, npair * P], I16, kind="ExternalInput").ap()
    iotarep_d = nc.dram_tensor("iotarep", [P, K, J1], I16, kind="ExternalInput").ap()
    iotacol2_d = nc.dram_tensor("iotacol2", [P, 1], F32, kind="ExternalInput").ap()
    id128_d = nc.dram_tensor("id128", [P, P], BF16, kind="ExternalInput").ap()
    id33_d = nc.dram_tensor("id33", [D + 1, D + 1], F32, kind="ExternalInput").ap()
    id64_d = nc.dram_tensor("id64", [K, K], F32, kind="ExternalInput").ap()
    eyeneg_d = nc.dram_tensor("eyeneg", [K, K], BF16, kind="ExternalInput").ap()
    out_d = nc.dram_tensor("out", [3], F32, kind="ExternalOutput").ap()

    n_chunks1 = math.ceil(tpc / J1)
    n_oc = math.ceil(npair / TPAIR)

    with tile.TileContext(nc, num_cores=N_CORES) as tc, ExitStack() as ctx:
        singles = ctx.enter_context(tc.tile_pool(name="singles", bufs=1))
        xpool = ctx.enter_context(tc.tile_pool(name="xpool", bufs=1))
        h2pool = ctx.enter_context(tc.tile_pool(name="h2pool", bufs=3))
        l2pool = ctx.enter_context(tc.tile_pool(name="l2pool", bufs=4))
        htpool = ctx.enter_context(tc.tile_pool(name="htpool", bufs=7))
        sqpool = ctx.enter_context(tc.tile_pool(name="sqpool", bufs=3))
        wpool = ctx.enter_context(tc.tile_pool(name="wpool", bufs=2))
        psA = ctx.enter_context(tc.tile_pool(name="psA", bufs=1, space="PSUM"))
        psMg = ctx.enter_context(tc.tile_pool(name="psMg", bufs=4, space="PSUM"))
        psS = ctx.enter_context(tc.tile_pool(name="psS", bufs=2, space="PSUM"))
        dram = ctx.enter_context(tc.tile_pool(name="dram", bufs=2, space="DRAM"))

        # ---------- constants ----------
        lpm = singles.tile([P, tpc], I16)
        nc.sync.dma_start(out=lpm, in_=lpm_d)
        iotarep = singles.tile([P, K, J1], I16)
        nc.sync.dma_start(out=iotarep, in_=iotarep_d)
        iotacol2 = singles.tile([P, 1], F32)
        nc.sync.dma_start(out=iotacol2, in_=iotacol2_d)
        id128 = singles.tile([P, P], BF16)
        nc.sync.dma_start(out=id128, in_=id128_d)
        id33 = singles.tile([D + 1, D + 1], F32)
        nc.sync.dma_start(out=id33, in_=id33_d)
        id64 = singles.tile([K, K], F32)
        nc.sync.dma_start(out=id64, in_=id64_d)
        eyeneg = singles.tile([K, K], BF16)
        nc.sync.dma_start(out=eyeneg, in_=eyeneg_d)
        d2all = singles.tile([P, tpc], F32)
        invc_all = singles.tile([P, tpc], BF16)

        # l2 chunk DMA helper (host-materialized paired label broadcast)
        def issue_l2(oc):
            t0 = oc * TPAIR
            tn = min(TPAIR, npair - t0)
            l2 = l2pool.tile([P, TPAIR * P], I16, tag="l2")
            nc.sync.dma_start(
                out=l2[:, : tn * P], in_=l2f_d[:, t0 * P : (t0 + tn) * P]
            )
            return l2, tn

        # ---------- pass 1: segment sums ----------
        # xe DMAs issued first so pass-1 is never starved by the (large)
        # l2 broadcast transfers; only a few l2 chunks go early to warm the
        # ht pipeline, the rest are issued after the AllReduce input DMA so
        # the collective is not queued behind them on the DMA engines.
        psumS = psA.tile([D + 1, K], F32)
        l2_tiles = []
        lc = 0
        t_done = 0
        xe = xpool.tile([P, tpc, D + 1], BF16)
        for c in range(n_chunks1):
            j0 = c * J1
            jn = min(J1, tpc - j0)
            nc.sync.dma_start(
                out=xe[:, j0 : j0 + jn, :], in_=xe_d[:, j0 : j0 + jn, :]
            )
            h2 = h2pool.tile([P, K, J1], BF16, tag="h2")
            nc.vector.tensor_tensor(
                h2[:, :, :jn],
                lpm[:, None, j0 : j0 + jn].to_broadcast((P, K, jn)),
                iotarep[:, :, :jn],
                mybir.AluOpType.is_equal,
            )
            for j in range(jn):
                nc.tensor.matmul(
                    psumS,
                    xe[:, j0 + j, :],
                    h2[:, :, j],
                    start=(t_done == 0),
                    stop=(t_done == tpc - 1),
                )
                t_done += 1
        # l2 label chunks stream right behind xe on the DMA engines
        while lc < n_oc:
            l2_tiles.append(issue_l2(lc))
            lc += 1
        # ---------- AllGather segment sums + local reduce ----------
        # (AllGather avoids the cost model's 1.875x AllReduce penalty; the
        #  8-way sum is 3 cheap tree adds done locally)
        sg_local = wpool.tile([D + 1, K], F32, tag="sg")
        nc.scalar.copy(out=sg_local, in_=psumS)
        cc_in = dram.tile([D + 1, K], F32)
        cc_out = dram.tile([N_CORES, D + 1, K], F32)
        nc.gpsimd.dma_start(out=cc_in, in_=sg_local)
        nc.gpsimd.collective_compute(
            "AllGather",
            mybir.AluOpType.bypass,
            replica_groups=[core_ids],
            ins=[cc_in.opt()],
            outs=[cc_out.opt()],
        )
        sg8 = wpool.tile([D + 1, N_CORES, K], F32, tag="sg8")
        ccf = cc_out[0, 0, 0]  # base AP for offset/tensor
        nc.gpsimd.dma_start(
            out=sg8,
            in_=bass.AP(
                tensor=ccf.tensor, offset=ccf.offset,
                ap=[[K, D + 1], [(D + 1) * K, N_CORES], [1, K]],
            ),
        )
        sg4 = wpool.tile([D + 1, 4, K], F32, tag="sg4")
        nc.vector.tensor_add(sg4, sg8[:, :4, :], sg8[:, 4:, :])
        sg2t = wpool.tile([D + 1, 2, K], F32, tag="sg2t")
        nc.vector.tensor_add(sg2t, sg4[:, :2, :], sg4[:, 2:, :])
        sg = wpool.tile([D + 1, K], F32, tag="sg2")
        nc.vector.tensor_tensor(
            sg, sg2t[:, 0, :], sg2t[:, 1, :], mybir.AluOpType.add
        )

        # ---------- ht builds (no AR dependency) ----------
        ht_tiles = []
        for oc in range(n_oc):
            l2, tn = l2_tiles[oc]
            ht = htpool.tile([P, TPAIR * P], BF16, tag="ht")
            nc.vector.tensor_single_scalar(
                ht[:, : tn * P], l2[:, : tn * P], iotacol2,
                mybir.AluOpType.is_equal,
            )
            ht_tiles.append(ht)

        # ---------- stats (Act/Pool/PE only; DVE stays on one-hot work) ----
        psW = psS.tile([K, D + 1], F32, tag="small")
        nc.tensor.transpose(psW, sg, id33)
        W = wpool.tile([K, D + 1], F32, tag="w")  # [S_k | c_k]
        nc.scalar.copy(out=W, in_=psW)
        safec = wpool.tile([K, 1], F32, tag="safec")
        nc.gpsimd.tensor_scalar(
            safec, W[:, D : D + 1], 256.0, 1.0,
            mybir.AluOpType.mult, mybir.AluOpType.max,
        )
        invc = wpool.tile([K, 1], F32, tag="invc")
        nc.vector.reciprocal(invc, safec)
        svp = wpool.tile([K, 1], F32, tag="svp")  # sqrt(1/c)
        nc.scalar.activation(
            out=svp, in_=invc, func=mybir.ActivationFunctionType.Sqrt
        )
        mu = wpool.tile([K, D], F32, tag="mu")
        nc.gpsimd.tensor_mul(mu, W[:, :D], invc.to_broadcast((K, D)))
        # table2 [128, 33] bf16 = [eps - mu | sqrt(1/c) - 1], rows replicated
        table2 = singles.tile([P, D + 1], BF16)
        nc.scalar.activation(
            out=table2[:K, :D], in_=mu,
            func=mybir.ActivationFunctionType.Copy, bias=EPS, scale=-1.0,
        )
        nc.scalar.activation(
            out=table2[:K, D : D + 1], in_=svp,
            func=mybir.ActivationFunctionType.Copy, bias=-1.0 / 256.0,
        )
        nc.sync.dma_start(out=table2[K:, :], in_=table2[:K, :])

        # ---------- inter + reg losses (Act/Pool/PE) ----------
        mup = wpool.tile([K, D], F32, tag="mup")  # mu + eps
        nc.scalar.activation(
            out=mup, in_=mu, func=mybir.ActivationFunctionType.Copy, bias=EPS
        )
        qsc = wpool.tile([K, D], F32, tag="qsc")
        nc.gpsimd.tensor_mul(qsc, mu, mu)
        q = wpool.tile([K, 1], F32, tag="q")  # ||mu||^2
        nc.vector.tensor_reduce(
            out=q, in_=qsc, axis=mybir.AxisListType.X, op=mybir.AluOpType.add
        )
        qpsc = wpool.tile([K, D], F32, tag="qpsc")
        nc.gpsimd.tensor_mul(qpsc, mup, mup)
        qp = wpool.tile([K, 1], F32, tag="qp")  # ||mu + eps||^2
        nc.vector.tensor_reduce(
            out=qp, in_=qpsc, axis=mybir.AxisListType.X, op=mybir.AluOpType.add
        )
        # pd2[a,b] = qp_a - 2*mup_a.mu_b + q_b via [ -2*mup | qp | 1 ] x [ mu | 1 | q ]
        ab = wpool.tile([K, D + 2], F32, tag="ab")
        nc.scalar.mul(out=ab[:, :D], in_=mup, mul=-2.0)
        nc.scalar.copy(out=ab[:, D : D + 1], in_=qp)
        nc.gpsimd.memset(ab[:, D + 1 : D + 2], 1.0)
        bb = wpool.tile([K, D + 2], F32, tag="bb")
        nc.scalar.copy(out=bb[:, :D], in_=mu)
        nc.gpsimd.memset(bb[:, D : D + 1], 1.0)
        nc.scalar.copy(out=bb[:, D + 1 : D + 2], in_=q)
        psT = psS.tile([D + 2, K], F32, tag="small")
        nc.tensor.transpose(psT, ab, id64)
        atp = wpool.tile([D + 2, K], F32, tag="atp")
        nc.scalar.copy(out=atp, in_=psT)
        psT2 = psS.tile([D + 2, K], F32, tag="small")
        nc.tensor.transpose(psT2, bb, id64)
        btp = wpool.tile([D + 2, K], F32, tag="btp")
        nc.scalar.copy(out=btp, in_=psT2)
        psPD = psS.tile([K, K], F32, tag="small")
        nc.tensor.matmul(psPD, atp, btp)
        pdc = wpool.tile([K, K], F32, tag="pdc")
        nc.vector.tensor_scalar_max(pdc, psPD, 0.0)
        pdist = wpool.tile([K, K], F32, tag="pdist")
        nc.scalar.activation(
            out=pdist, in_=pdc, func=mybir.ActivationFunctionType.Sqrt
        )
        hingeI = wpool.tile([K, K], F32, tag="hingeI")
        nc.scalar.activation(
            out=hingeI, in_=pdist, func=mybir.ActivationFunctionType.Relu,
            bias=float(INTER_MARGIN2), scale=-1.0,
        )
        hm = wpool.tile([K, K], F32, tag="hm")
        nc.gpsimd.tensor_mul(hm, hingeI, eyeneg)
        hm2 = wpool.tile([K, K], F32, tag="hm2")
        nc.gpsimd.tensor_mul(hm2, hm, hm)
        interp = wpool.tile([K, 1], F32, tag="interp")
        nc.vector.tensor_reduce(
            out=interp, in_=hm2, axis=mybir.AxisListType.X,
            op=mybir.AluOpType.add,
        )
        sqp = wpool.tile([K, 1], F32, tag="sqp")  # ||mu + eps||
        nc.scalar.activation(
            out=sqp, in_=qp, func=mybir.ActivationFunctionType.Sqrt
        )
        cat2 = wpool.tile([K, 2], F32, tag="cat2")
        nc.scalar.copy(out=cat2[:, 0:1], in_=interp)
        nc.scalar.copy(out=cat2[:, 1:2], in_=sqp)
        ones64 = singles.tile([K, 1], F32)
        nc.gpsimd.memset(ones64, 1.0)
        psIR = psS.tile([1, 2], F32, tag="small")
        nc.tensor.matmul(psIR, ones64, cat2)
        ir = wpool.tile([1, 2], F32, tag="ir")  # [inter_sum, reg_sum]
        nc.scalar.copy(out=ir, in_=psIR)

        # ---------- pass 2: gather + diff in PSUM, square, fold-reduce ------
        for oc in range(n_oc):
            ht = ht_tiles[oc]
            pc = min(TPAIR, npair - oc * TPAIR)
            cbase = oc * TPAIR * 2        # first global tile of this chunk
            ctn = min(pc * 2, tpc - cbase)  # tiles in this chunk
            # one sq tile per ht chunk (up to 56 tiles), 4 PSUM groups
            sq = sqpool.tile([P, TPAIR * 2, D + 1], BF16, tag="sq")
            for g in range(math.ceil(pc / (JMG // 2))):
                p0 = g * (JMG // 2)
                pn = min(JMG // 2, pc - p0)
                jbase = (oc * TPAIR + p0) * 2  # first global tile of group
                nt = min(pn * 2, tpc - jbase)
                psD = psMg.tile([P, JMG, D + 1], F32, tag="psd")
                for lp in range(pn):
                    for half in range(2):
                        t = lp * 2 + half
                        if t >= nt:
                            break
                        colp = p0 + lp
                        nc.tensor.matmul(
                            psD[:, t, :],
                            ht[64 * half : 64 * (half + 1),
                               colp * P : (colp + 1) * P],
                            table2[64 * half : 64 * (half + 1), :],
                            start=True, stop=False,
                        )
                        nc.tensor.matmul(
                            psD[:, t, :], id128,
                            xe[:, cbase + p0 * 2 + t, :],
                            start=False, stop=True,
                        )
                t0 = p0 * 2
                nc.scalar.activation(
                    out=sq[:, t0 : t0 + nt, :], in_=psD[:, :nt, :],
                    func=mybir.ActivationFunctionType.Square,
                )
            # bf16 fold-tree reduce over D (2x DVE mode); f1 of every other
            # chunk goes to Pool to offload DVE
            with nc.allow_low_precision(reason="bf16 partial sums of d2"):
                f1 = sqpool.tile([P, TPAIR * 2, 16], BF16, tag="f1")
                f1eng = nc.gpsimd if (oc % 2 == 1) else nc.vector
                f1eng.tensor_add(
                    f1[:, :ctn, :], sq[:, :ctn, 0:16], sq[:, :ctn, 16:32]
                )
                f2 = sqpool.tile([P, TPAIR * 2, 8], BF16, tag="f2")
                nc.vector.tensor_add(
                    f2[:, :ctn, :], f1[:, :ctn, 0:8], f1[:, :ctn, 8:16]
                )
                f3 = sqpool.tile([P, TPAIR * 2, 4], BF16, tag="f3")
                nc.vector.tensor_add(
                    f3[:, :ctn, :], f2[:, :ctn, 0:4], f2[:, :ctn, 4:8]
                )
                f4 = sqpool.tile([P, TPAIR * 2, 2], BF16, tag="f4")
                nc.vector.tensor_add(
                    f4[:, :ctn, :], f3[:, :ctn, 0:2], f3[:, :ctn, 2:4]
                )
            nc.vector.tensor_tensor(
                d2all[:, cbase : cbase + ctn],
                f4[:, :ctn, 0], f4[:, :ctn, 1], mybir.AluOpType.add,
            )
            nc.gpsimd.tensor_scalar_add(
                invc_all[:, cbase : cbase + ctn], sq[:, :ctn, D], 0.0
            )

        # ---------- finals ----------
        nc.scalar.activation(
            out=d2all, in_=d2all, func=mybir.ActivationFunctionType.Sqrt
        )
        margneg = singles.tile([P, 1], F32)
        nc.gpsimd.memset(margneg, -float(INTRA_MARGIN))
        nc.scalar.activation(
            out=d2all, in_=d2all, func=mybir.ActivationFunctionType.Relu,
            bias=margneg,
        )
        hh = singles.tile([P, tpc], F32)
        nc.gpsimd.tensor_mul(hh, d2all, d2all)
        hhw = singles.tile([P, tpc], F32)
        nc.gpsimd.tensor_mul(hhw, hh, invc_all)
        rowsum = singles.tile([P, 1], F32)
        nc.vector.tensor_reduce(
            out=rowsum, in_=hhw, axis=mybir.AxisListType.X,
            op=mybir.AluOpType.add,
        )
        ones128 = singles.tile([P, 1], F32)
        nc.gpsimd.memset(ones128, 1.0)
        psL = psS.tile([1, 1], F32, tag="small")
        nc.tensor.matmul(psL, rowsum, ones128)
        tot = wpool.tile([1, 3], F32, tag="tot")
        nc.scalar.copy(out=tot[:, 0:1], in_=psL)
        nc.scalar.copy(out=tot[:, 1:3], in_=ir)
        nc.sync.dma_start(out=out_d, in_=tot[0:1, :])

    nc.compile()
    return nc


_NC_CACHE = {}


def _get_program(tpc):
    if tpc not in _NC_CACHE:
        _NC_CACHE[tpc] = build_program(tpc)
    return _NC_CACHE[tpc]


def kernel(features, labels, num_clusters):
    features = np.asarray(features)
    labels = np.asarray(labels)
    n_total = features.shape[0]
    n_core = n_total // N_CORES
    tpc = math.ceil(n_core / P)
    nc = _get_program(tpc)
    in_maps = _host_prep(features, labels, tpc)
    res = run_bass_kernel_spmd(nc, in_maps, list(range(N_CORES)))
    intra_sum = sum(float(res.results[c]["out"][0]) for c in range(N_CORES))
    inter_sum = float(res.results[0]["out"][1])
    reg_sum = float(res.results[0]["out"][2])
    total = (
        intra_sum / K
        + inter_sum / (K * (K - 1))
        + 0.001 * reg_sum / K
    )
    return np.float32(total)


# revision 61
# speedup vs baseline: 1.7282x; 1.0480x over previous
# kernel.py — DiscriminativeLoss on 8 TRN2 NeuronCores (Bass/Tile, SPMD).
#
# Math (matches reference):
#   counts_k = #{i: l_i = k};  S_k = sum_{i in k} x_i;  mu_k = S_k / max(c_k, 1)
#   intra = (1/K) * sum_i invc_{l_i} * relu(||x_i - mu_{l_i} + eps|| - 1.5)^2
#   inter = sum_{a != b} relu(1 - ||(mu_a + eps) - mu_b||)^2 / (K*(K-1))
#   reg   = (1/K) * sum_k ||mu_k + eps||
#   total = intra + inter + 0.001 * reg
#
# V2 design (engine-balanced, cost-model driven):
#   pass 1: one-hot H2 [P, K, jn] via DVE/Pool tensor_tensor is_equal (2x
#     mode: all operands 2-byte packed SBUF); PE matmul lhsT=Xe [128,33],
#     rhs=H2[:, :, j] accumulates S^T = [S | counts] in PSUM [33, 64].
#   AllReduce [33, 64]; stats (mu, 1/c, sqrt(1/c), inter/reg losses) on
#     Act/Pool/PE only, keeping DVE free.
#   pass 2: paired transposed one-hot ht [128, 128] per tile-pair (A on
#     partitions 0-63, B on 64-127) built from broadcast-DMA'd labels via
#     DVE tensor_single_scalar is_equal (4x mode). Per tile, TWO accumulating
#     matmuls produce diff = x - (mu - eps) directly in PSUM:
#       psD  = ht_half^T @ [eps - mu | sqrt(1/c) - 1]   (gather, negated)
#       psD += I_128    @ Xe_tile                        ([x | 1])
#     -> psD = [x - mu + eps | sqrt(1/c)].
#   Act Square psD -> sq bf16 (col 32 squares to 1/c); DVE/Pool tensor_reduce
#     over D -> d2; DVE copies col 32 -> invc_all.
#   finals: dist=sqrt(d2), h=relu(dist-1.5) on Act; intra partial
#     sum_i h^2 * invc via DVE mults + tensor_tensor_reduce + PE ones-matmul.
import math
import numpy as np
from contextlib import ExitStack

import concourse.bass as bass
import concourse.bacc as bacc
import concourse.tile as tile
import concourse.mybir as mybir
from concourse.bass_utils import run_bass_kernel_spmd

F32 = mybir.dt.float32
BF16 = mybir.dt.bfloat16
I16 = mybir.dt.int16

N_CORES = 8
K = 64
D = 32
P = 128
EPS = 1e-8
PAD_LABEL = 999  # never matches any one-hot column

INTRA_MARGIN = 1.5
INTER_MARGIN2 = 1.0  # 2 * 0.5

J1 = 20      # pass-1 chunk width (tiles)
TPAIR = 28   # tile-pairs per ht chunk (4 gather groups of 7 pairs)
JMG = 14     # tiles per PSUM gather group (7 pairs)


def _host_prep(features, labels, tpc):
    """Shard + relayout on host. Returns per-core input dicts."""
    n_total = features.shape[0]
    n_core = n_total // N_CORES
    n_pad = P * tpc
    npair = (tpc + 1) // 2
    import ml_dtypes

    in_maps = []
    for c in range(N_CORES):
        f = np.asarray(features[c * n_core : (c + 1) * n_core], dtype=np.float32)
        l = np.asarray(labels[c * n_core : (c + 1) * n_core], dtype=np.int64)
        if n_pad > n_core:
            f = np.concatenate([f, np.zeros((n_pad - n_core, D), np.float32)], axis=0)
            l = np.concatenate([l, np.full((n_pad - n_core,), PAD_LABEL, np.int64)])
        # Xe: [P, tpc, 33] bf16, col 32 = 1/256 (exact in bf16; keeps the
        # sqrt(1/c) gather free of bf16 cancellation); point i = (i%P, i//P)
        xe = np.full((n_pad, D + 1), 1.0 / 256.0, np.float32)
        xe[:, :D] = f
        xe = xe.reshape(P, tpc, D + 1).astype(ml_dtypes.bfloat16)
        lpj = l.reshape(P, tpc)  # [point-in-tile, tile]
        l_pm = lpj.astype(np.int16)
        # paired label broadcast for ht, fully materialized on host:
        # [128, npair*128] int16, rows 0-63 = labels of tile 2jj, rows
        # 64-127 = labels of tile 2jj+1 (one contiguous DMA per chunk)
        ltm = lpj.T.astype(np.int16)  # [tpc, P]
        l_tma = np.full((npair, P), PAD_LABEL, np.int16)
        l_tmb = np.full((npair, P), PAD_LABEL, np.int16)
        l_tma[:] = ltm[0::2]
        nb = tpc // 2
        l_tmb[:nb] = ltm[1::2]
        l2full = np.empty((P, npair * P), np.int16)
        l2full[:K] = np.broadcast_to(
            l_tma.reshape(1, npair * P), (K, npair * P)
        )
        l2full[K:] = np.broadcast_to(
            l_tmb.reshape(1, npair * P), (K, npair * P)
        )
        # iotarep [P, K, J1] int16: value k at [:, k, :]
        iotarep = np.tile(
            np.arange(K, dtype=np.int16)[None, :, None], (P, 1, J1)
        )
        in_maps.append(
            {
                "xe": np.ascontiguousarray(xe),
                "labels_pm": np.ascontiguousarray(l_pm),
                "l2full": l2full,
                "iotarep": np.ascontiguousarray(iotarep),
                "iotacol2": np.concatenate(
                    [np.arange(K), np.arange(K)]
                ).astype(np.float32).reshape(P, 1),
                "id128": np.eye(P, dtype=ml_dtypes.bfloat16),
                "id33": np.eye(D + 1, dtype=np.float32),
                "id64": np.eye(K, dtype=np.float32),
                "eyeneg": (1.0 - np.eye(K, dtype=np.float32)).astype(
                    ml_dtypes.bfloat16
                ),
            }
        )
    return in_maps


def build_program(tpc, dve_sq_every=6, f1_dve_every=0, ht_bufs=8, l2_bufs=3, mg_bufs=4):
    """Build the SPMD Bass program. tpc = tiles per core."""
    nc = bacc.Bacc(
        "TRN2", target_bir_lowering=False, debug=False, num_devices=N_CORES
    )
    core_ids = list(range(N_CORES))
    npair = (tpc + 1) // 2

    xe_d = nc.dram_tensor("xe", [P, tpc, D + 1], BF16, kind="ExternalInput").ap()
    lpm_d = nc.dram_tensor("labels_pm", [P, tpc], I16, kind="ExternalInput").ap()
    l2f_d = nc.dram_tensor("l2full", [P, npair * P], I16, kind="ExternalInput").ap()
    iotarep_d = nc.dram_tensor("iotarep", [P, K, J1], I16, kind="ExternalInput").ap()
    iotacol2_d = nc.dram_tensor("iotacol2", [P, 1], F32, kind="ExternalInput").ap()
    id128_d = nc.dram_tensor("id128", [P, P], BF16, kind="ExternalInput").ap()
    id33_d = nc.dram_tensor("id33", [D + 1, D + 1], F32, kind="ExternalInput").ap()
    id64_d = nc.dram_tensor("id64", [K, K], F32, kind="ExternalInput").ap()
    eyeneg_d = nc.dram_tensor("eyeneg", [K, K], BF16, kind="ExternalInput").ap()
    out_d = nc.dram_tensor("out", [3], F32, kind="ExternalOutput").ap()

    n_chunks1 = math.ceil(tpc / J1)
    n_oc = math.ceil(npair / TPAIR)

    with tile.TileContext(nc, num_cores=N_CORES) as tc, ExitStack() as ctx:
        singles = ctx.enter_context(tc.tile_pool(name="singles", bufs=1))
        xpool = ctx.enter_context(tc.tile_pool(name="xpool", bufs=1))
        h2pool = ctx.enter_context(tc.tile_pool(name="h2pool", bufs=3))
        l2pool = ctx.enter_context(tc.tile_pool(name="l2pool", bufs=l2_bufs))
        htpool = ctx.enter_context(tc.tile_pool(name="htpool", bufs=ht_bufs))
        sqpool = ctx.enter_context(tc.tile_pool(name="sqpool", bufs=3))
        wpool = ctx.enter_context(tc.tile_pool(name="wpool", bufs=2))
        psA = ctx.enter_context(tc.tile_pool(name="psA", bufs=1, space="PSUM"))
        psMg = ctx.enter_context(tc.tile_pool(name="psMg", bufs=mg_bufs, space="PSUM"))
        psS = ctx.enter_context(tc.tile_pool(name="psS", bufs=2, space="PSUM"))
        dram = ctx.enter_context(tc.tile_pool(name="dram", bufs=2, space="DRAM"))

        # ---------- constants ----------
        lpm = singles.tile([P, tpc], I16)
        nc.sync.dma_start(out=lpm, in_=lpm_d)
        iotarep = singles.tile([P, K, J1], I16)
        nc.sync.dma_start(out=iotarep, in_=iotarep_d)
        iotacol2 = singles.tile([P, 1], F32)
        nc.sync.dma_start(out=iotacol2, in_=iotacol2_d)
        id128 = singles.tile([P, P], BF16)
        nc.sync.dma_start(out=id128, in_=id128_d)
        id33 = singles.tile([D + 1, D + 1], F32)
        nc.sync.dma_start(out=id33, in_=id33_d)
        id64 = singles.tile([K, K], F32)
        nc.sync.dma_start(out=id64, in_=id64_d)
        eyeneg = singles.tile([K, K], BF16)
        nc.sync.dma_start(out=eyeneg, in_=eyeneg_d)
        d2all = singles.tile([P, tpc], F32)
        invc_all = singles.tile([P, tpc], BF16)
        hh = singles.tile([P, tpc], F32)
        hhw = singles.tile([P, tpc], F32)
        rsacc = singles.tile([P, n_oc], F32)
        margneg = singles.tile([P, 1], F32)
        nc.gpsimd.memset(margneg, -float(INTRA_MARGIN))

        # l2 chunk DMA helper (host-materialized paired label broadcast)
        def issue_l2(oc):
            t0 = oc * TPAIR
            tn = min(TPAIR, npair - t0)
            l2 = l2pool.tile([P, TPAIR * P], I16, tag="l2")
            nc.sync.dma_start(
                out=l2[:, : tn * P], in_=l2f_d[:, t0 * P : (t0 + tn) * P]
            )
            return l2, tn

        # ---------- pass 1: segment sums ----------
        # xe DMAs issued first so pass-1 is never starved by the (large)
        # l2 broadcast transfers; only a few l2 chunks go early to warm the
        # ht pipeline, the rest are issued after the AllReduce input DMA so
        # the collective is not queued behind them on the DMA engines.
        psumS = psA.tile([D + 1, K], F32)
        l2_tiles = []
        lc = 0
        t_done = 0
        xe = xpool.tile([P, tpc, D + 1], BF16)
        for c in range(n_chunks1):
            j0 = c * J1
            jn = min(J1, tpc - j0)
            nc.sync.dma_start(
                out=xe[:, j0 : j0 + jn, :], in_=xe_d[:, j0 : j0 + jn, :]
            )
            h2 = h2pool.tile([P, K, J1], BF16, tag="h2")
            nc.vector.tensor_tensor(
                h2[:, :, :jn],
                lpm[:, None, j0 : j0 + jn].to_broadcast((P, K, jn)),
                iotarep[:, :, :jn],
                mybir.AluOpType.is_equal,
            )
            for j in range(jn):
                nc.tensor.matmul(
                    psumS,
                    xe[:, j0 + j, :],
                    h2[:, :, j],
                    start=(t_done == 0),
                    stop=(t_done == tpc - 1),
                )
                t_done += 1
        # l2 label chunks stream right behind xe on the DMA engines
        while lc < n_oc:
            l2_tiles.append(issue_l2(lc))
            lc += 1
        # ---------- AllGather segment sums + local reduce ----------
        # (AllGather avoids the cost model's 1.875x AllReduce penalty; the
        #  8-way sum is 3 cheap tree adds done locally)
        sg_local = wpool.tile([D + 1, K], F32, tag="sg")
        nc.scalar.copy(out=sg_local, in_=psumS)
        cc_in = dram.tile([D + 1, K], F32)
        cc_out = dram.tile([N_CORES, D + 1, K], F32)
        nc.gpsimd.dma_start(out=cc_in, in_=sg_local)
        nc.gpsimd.collective_compute(
            "AllGather",
            mybir.AluOpType.bypass,
            replica_groups=[core_ids],
            ins=[cc_in.opt()],
            outs=[cc_out.opt()],
        )
        sg8 = wpool.tile([D + 1, N_CORES, K], F32, tag="sg8")
        ccf = cc_out[0, 0, 0]  # base AP for offset/tensor
        nc.gpsimd.dma_start(
            out=sg8,
            in_=bass.AP(
                tensor=ccf.tensor, offset=ccf.offset,
                ap=[[K, D + 1], [(D + 1) * K, N_CORES], [1, K]],
            ),
        )
        sg4 = wpool.tile([D + 1, 4, K], F32, tag="sg4")
        nc.vector.tensor_add(sg4, sg8[:, :4, :], sg8[:, 4:, :])
        sg2t = wpool.tile([D + 1, 2, K], F32, tag="sg2t")
        nc.vector.tensor_add(sg2t, sg4[:, :2, :], sg4[:, 2:, :])
        sg = wpool.tile([D + 1, K], F32, tag="sg2")
        nc.vector.tensor_tensor(
            sg, sg2t[:, 0, :], sg2t[:, 1, :], mybir.AluOpType.add
        )

        # ---------- ht builds (no AR dependency) ----------
        ht_tiles = []
        for oc in range(n_oc):
            l2, tn = l2_tiles[oc]
            ht = htpool.tile([P, TPAIR * P], BF16, tag="ht")
            nc.vector.tensor_single_scalar(
                ht[:, : tn * P], l2[:, : tn * P], iotacol2,
                mybir.AluOpType.is_equal,
            )
            ht_tiles.append(ht)

        # ---------- stats (Act/Pool/PE only; DVE stays on one-hot work) ----
        psW = psS.tile([K, D + 1], F32, tag="small")
        nc.tensor.transpose(psW, sg, id33)
        W = wpool.tile([K, D + 1], F32, tag="w")  # [S_k | c_k]
        nc.scalar.copy(out=W, in_=psW)
        safec = wpool.tile([K, 1], F32, tag="safec")
        nc.gpsimd.tensor_scalar(
            safec, W[:, D : D + 1], 256.0, 1.0,
            mybir.AluOpType.mult, mybir.AluOpType.max,
        )
        invc = wpool.tile([K, 1], F32, tag="invc")
        nc.vector.reciprocal(invc, safec)
        svp = wpool.tile([K, 1], F32, tag="svp")  # sqrt(1/c)
        nc.scalar.activation(
            out=svp, in_=invc, func=mybir.ActivationFunctionType.Sqrt
        )
        mu = wpool.tile([K, D], F32, tag="mu")
        nc.gpsimd.tensor_mul(mu, W[:, :D], invc.to_broadcast((K, D)))
        # table2 [128, 33] bf16 = [eps - mu | sqrt(1/c) - 1], rows replicated
        table2 = singles.tile([P, D + 1], BF16)
        nc.scalar.activation(
            out=table2[:K, :D], in_=mu,
            func=mybir.ActivationFunctionType.Copy, bias=EPS, scale=-1.0,
        )
        nc.scalar.activation(
            out=table2[:K, D : D + 1], in_=svp,
            func=mybir.ActivationFunctionType.Copy, bias=-1.0 / 256.0,
        )
        nc.sync.dma_start(out=table2[K:, :], in_=table2[:K, :])

        # ---------- inter + reg losses (Act/Pool/PE) ----------
        mup = wpool.tile([K, D], F32, tag="mup")  # mu + eps
        nc.scalar.activation(
            out=mup, in_=mu, func=mybir.ActivationFunctionType.Copy, bias=EPS
        )
        qsc = wpool.tile([K, D], F32, tag="qsc")
        nc.gpsimd.tensor_mul(qsc, mu, mu)
        q = wpool.tile([K, 1], F32, tag="q")  # ||mu||^2
        nc.vector.tensor_reduce(
            out=q, in_=qsc, axis=mybir.AxisListType.X, op=mybir.AluOpType.add
        )
        qpsc = wpool.tile([K, D], F32, tag="qpsc")
        nc.gpsimd.tensor_mul(qpsc, mup, mup)
        qp = wpool.tile([K, 1], F32, tag="qp")  # ||mu + eps||^2
        nc.vector.tensor_reduce(
            out=qp, in_=qpsc, axis=mybir.AxisListType.X, op=mybir.AluOpType.add
        )
        # pd2[a,b] = qp_a - 2*mup_a.mu_b + q_b via [ -2*mup | qp | 1 ] x [ mu | 1 | q ]
        ab = wpool.tile([K, D + 2], F32, tag="ab")
        nc.scalar.mul(out=ab[:, :D], in_=mup, mul=-2.0)
        nc.scalar.copy(out=ab[:, D : D + 1], in_=qp)
        nc.gpsimd.memset(ab[:, D + 1 : D + 2], 1.0)
        bb = wpool.tile([K, D + 2], F32, tag="bb")
        nc.scalar.copy(out=bb[:, :D], in_=mu)
        nc.gpsimd.memset(bb[:, D : D + 1], 1.0)
        nc.scalar.copy(out=bb[:, D + 1 : D + 2], in_=q)
        psT = psS.tile([D + 2, K], F32, tag="small")
        nc.tensor.transpose(psT, ab, id64)
        atp = wpool.tile([D + 2, K], F32, tag="atp")
        nc.scalar.copy(out=atp, in_=psT)
        psT2 = psS.tile([D + 2, K], F32, tag="small")
        nc.tensor.transpose(psT2, bb, id64)
        btp = wpool.tile([D + 2, K], F32, tag="btp")
        nc.scalar.copy(out=btp, in_=psT2)
        psPD = psS.tile([K, K], F32, tag="small")
        nc.tensor.matmul(psPD, atp, btp)
        pdc = wpool.tile([K, K], F32, tag="pdc")
        nc.vector.tensor_scalar_max(pdc, psPD, 0.0)
        pdist = wpool.tile([K, K], F32, tag="pdist")
        nc.scalar.activation(
            out=pdist, in_=pdc, func=mybir.ActivationFunctionType.Sqrt
        )
        hingeI = wpool.tile([K, K], F32, tag="hingeI")
        nc.scalar.activation(
            out=hingeI, in_=pdist, func=mybir.ActivationFunctionType.Relu,
            bias=float(INTER_MARGIN2), scale=-1.0,
        )
        hm = wpool.tile([K, K], F32, tag="hm")
        nc.gpsimd.tensor_mul(hm, hingeI, eyeneg)
        hm2 = wpool.tile([K, K], F32, tag="hm2")
        nc.gpsimd.tensor_mul(hm2, hm, hm)
        interp = wpool.tile([K, 1], F32, tag="interp")
        nc.vector.tensor_reduce(
            out=interp, in_=hm2, axis=mybir.AxisListType.X,
            op=mybir.AluOpType.add,
        )
        sqp = wpool.tile([K, 1], F32, tag="sqp")  # ||mu + eps||
        nc.scalar.activation(
            out=sqp, in_=qp, func=mybir.ActivationFunctionType.Sqrt
        )
        cat2 = wpool.tile([K, 2], F32, tag="cat2")
        nc.scalar.copy(out=cat2[:, 0:1], in_=interp)
        nc.scalar.copy(out=cat2[:, 1:2], in_=sqp)
        ones64 = singles.tile([K, 1], F32)
        nc.gpsimd.memset(ones64, 1.0)
        psIR = psS.tile([1, 2], F32, tag="small")
        nc.tensor.matmul(psIR, ones64, cat2)
        ir = wpool.tile([1, 2], F32, tag="ir")  # [inter_sum, reg_sum]
        nc.scalar.copy(out=ir, in_=psIR)

        # ---------- pass 2: gather + diff in PSUM, square, fold-reduce ------
        fin_oc = sorted(set(
            [n_oc - 1] + [max(0, (n_oc * (q + 1)) // 4 - 1) for q in range(3)]
        ))
        fin_base = []
        prev = 0
        for oc_ in fin_oc:
            fin_base.append(prev)
            pc_ = min(TPAIR, npair - oc_ * TPAIR)
            prev = min(oc_ * TPAIR * 2 + pc_ * 2, tpc)
        for oc in range(n_oc):
            ht = ht_tiles[oc]
            pc = min(TPAIR, npair - oc * TPAIR)
            cbase = oc * TPAIR * 2        # first global tile of this chunk
            ctn = min(pc * 2, tpc - cbase)  # tiles in this chunk
            # one sq tile per ht chunk (up to 56 tiles), 4 PSUM groups
            sq = sqpool.tile([P, TPAIR * 2, D + 1], BF16, tag="sq")
            for g in range(math.ceil(pc / (JMG // 2))):
                p0 = g * (JMG // 2)
                pn = min(JMG // 2, pc - p0)
                jbase = (oc * TPAIR + p0) * 2  # first global tile of group
                nt = min(pn * 2, tpc - jbase)
                psD = psMg.tile([P, JMG, D + 1], F32, tag="psd")
                for lp in range(pn):
                    for half in range(2):
                        t = lp * 2 + half
                        if t >= nt:
                            break
                        colp = p0 + lp
                        nc.tensor.matmul(
                            psD[:, t, :],
                            ht[64 * half : 64 * (half + 1),
                               colp * P : (colp + 1) * P],
                            table2[64 * half : 64 * (half + 1), :],
                            start=True, stop=False,
                        )
                        nc.tensor.matmul(
                            psD[:, t, :], id128,
                            xe[:, cbase + p0 * 2 + t, :],
                            start=False, stop=True,
                        )
                t0 = p0 * 2
                if dve_sq_every and (oc * 4 + g) % dve_sq_every == dve_sq_every - 1:
                    # DVE square: PSUM copy then bf16 self-mult (one PSUM
                    # input per instruction as required by hardware)
                    cpy = sqpool.tile([P, JMG, D + 1], BF16, tag="cpy")
                    nc.vector.tensor_scalar_add(
                        cpy[:, :nt, :], psD[:, :nt, :], 0.0
                    )
                    nc.vector.tensor_mul(
                        sq[:, t0 : t0 + nt, :], cpy[:, :nt, :], cpy[:, :nt, :]
                    )
                else:
                    nc.scalar.activation(
                        out=sq[:, t0 : t0 + nt, :], in_=psD[:, :nt, :],
                        func=mybir.ActivationFunctionType.Square,
                    )
            # bf16 fold-tree reduce over D (2x DVE mode); f1 of every other
            # chunk goes to Pool to offload DVE
            with nc.allow_low_precision(reason="bf16 partial sums of d2"):
                f1 = sqpool.tile([P, TPAIR * 2, 16], BF16, tag="f1")
                f1eng = nc.vector if (f1_dve_every and oc % f1_dve_every == f1_dve_every - 1) else nc.gpsimd
                f1eng.tensor_add(
                    f1[:, :ctn, :], sq[:, :ctn, 0:16], sq[:, :ctn, 16:32]
                )
                f2 = sqpool.tile([P, TPAIR * 2, 8], BF16, tag="f2")
                nc.vector.tensor_add(
                    f2[:, :ctn, :], f1[:, :ctn, 0:8], f1[:, :ctn, 8:16]
                )
                f3 = sqpool.tile([P, TPAIR * 2, 4], BF16, tag="f3")
                nc.vector.tensor_add(
                    f3[:, :ctn, :], f2[:, :ctn, 0:4], f2[:, :ctn, 4:8]
                )
                f4 = sqpool.tile([P, TPAIR * 2, 2], BF16, tag="f4")
                nc.vector.tensor_add(
                    f4[:, :ctn, :], f3[:, :ctn, 0:2], f3[:, :ctn, 2:4]
                )
            nc.vector.tensor_tensor(
                d2all[:, cbase : cbase + ctn],
                f4[:, :ctn, 0], f4[:, :ctn, 1], mybir.AluOpType.add,
            )
            nc.gpsimd.tensor_scalar_add(
                invc_all[:, cbase : cbase + ctn], sq[:, :ctn, D], 0.0
            )
            # quarter-granularity finals (keeps the serial tail short
            # without flooding Act with per-chunk overhead):
            # dist = sqrt(d2); h = relu(dist - 1.5); acc_q = sum h^2 * invc
            if oc in fin_oc:
                qi = fin_oc.index(oc)
                b0 = fin_base[qi]
                b1 = cbase + ctn
                dsl = d2all[:, b0:b1]
                nc.scalar.activation(
                    out=dsl, in_=dsl,
                    func=mybir.ActivationFunctionType.Sqrt,
                )
                nc.scalar.activation(
                    out=dsl, in_=dsl,
                    func=mybir.ActivationFunctionType.Relu, bias=margneg,
                )
                hsl = hh[:, b0:b1]
                nc.vector.tensor_mul(hsl, dsl, dsl)
                wsl = hhw[:, b0:b1]
                nc.vector.tensor_mul(wsl, hsl, invc_all[:, b0:b1])
                nc.vector.tensor_reduce(
                    out=rsacc[:, qi : qi + 1], in_=wsl,
                    axis=mybir.AxisListType.X, op=mybir.AluOpType.add,
                )

        # ---------- finals: reduce per-quarter partials ----------
        rowsum = singles.tile([P, 1], F32)
        nc.vector.tensor_reduce(
            out=rowsum, in_=rsacc[:, : len(fin_oc)],
            axis=mybir.AxisListType.X, op=mybir.AluOpType.add,
        )
        ones128 = singles.tile([P, 1], F32)
        nc.gpsimd.memset(ones128, 1.0)
        psL = psS.tile([1, 1], F32, tag="small")
        nc.tensor.matmul(psL, rowsum, ones128)
        tot = wpool.tile([1, 3], F32, tag="tot")
        nc.scalar.copy(out=tot[:, 0:1], in_=psL)
        nc.scalar.copy(out=tot[:, 1:3], in_=ir)
        nc.sync.dma_start(out=out_d, in_=tot[0:1, :])

    nc.compile()
    return nc


_NC_CACHE = {}


def _get_program(tpc):
    if tpc not in _NC_CACHE:
        _NC_CACHE[tpc] = build_program(tpc)
    return _NC_CACHE[tpc]


def kernel(features, labels, num_clusters):
    features = np.asarray(features)
    labels = np.asarray(labels)
    n_total = features.shape[0]
    n_core = n_total // N_CORES
    tpc = math.ceil(n_core / P)
    nc = _get_program(tpc)
    in_maps = _host_prep(features, labels, tpc)
    res = run_bass_kernel_spmd(nc, in_maps, list(range(N_CORES)))
    intra_sum = sum(float(res.results[c]["out"][0]) for c in range(N_CORES))
    inter_sum = float(res.results[0]["out"][1])
    reg_sum = float(res.results[0]["out"][2])
    total = (
        intra_sum / K
        + inter_sum / (K * (K - 1))
        + 0.001 * reg_sum / K
    )
    return np.float32(total)
